# revision 35
# baseline (speedup 1.0000x reference)
"""Trainium2 Bass kernel for nn_DecoderLayer_90967407329666.

Decoder layer: LN1 -> QKV (+type emb) -> multi-axis RoPE -> causal SDPA
-> residual -> LN2 -> SwiGLU FFN -> residual.  B=2, T=2048, D=768, H=8,
DFF=2048, NTYPE=16, NAX=2 rotary axes of 32 dims each.

Sharding (8 cores):
  Phase 1 (token-parallel): each core owns 512 tokens (core c: batch c//4,
    tokens 512*(c%4)...) and computes LN1 + q,k (feature-major) + v
    (token-major) + type-emb + RoPE for those tokens, all 8 heads.
  Single AllToAll: block h carries head h's q,k,v (288 rows bf16) ->
    core c ends up with head c for ALL 4096 tokens.
  Phase 2 (head-parallel): core c runs full causal attention for head c,
    both batches; per-unit softmax normalization pipelined behind matmuls.
  AllToAll #2 (bf16): o goes back token-parallel.
  Phase 3 (token-parallel): residual + LN2 + SwiGLU FFN + residual.

dtypes: weights + exchange slabs in bf16; residual stream, LN stats and
position/angle math in fp32/fp32r (fp32r matmuls run at full PE rate for
free-dim >= 256, same as bf16).  LN gains/biases are folded into the
weights host-side (W' = g*W, bias row b@W folded into type-emb / biases).
A tiny AllToAll issued at t=0 absorbs the cross-core rendezvous skew.
"""

import sys

sys.path.insert(0, "/opt/trn_rl_repo")

import numpy as np
import ml_dtypes

import contextlib

import concourse.bacc as bacc
import concourse.bass as bass
import concourse.tile as tile
from concourse import mybir
from concourse.bass_utils import run_bass_kernel_spmd

BF16NP = np.dtype(ml_dtypes.bfloat16)

# ---- problem constants (hardcoded per contest rules) ----
B, T = 2, 2048
D, H, DFF, NTYPE = 768, 8, 2048, 16
NAX = 2
HD = D // H            # 96
DR = HD // (NAX + 1)   # 32
EPS = 1e-5
THETA = 10000.0
N_CORES = 8
TPC = 512              # tokens per core
NSUP = 4               # supertiles per batch (2048/512)
KD = D // 128          # 6 contraction chunks over D
SCALE = 1.0 / np.sqrt(np.float32(HD))

F32 = mybir.dt.float32
F32R = mybir.dt.float32r
BF16 = mybir.dt.bfloat16
I32 = mybir.dt.int32

# Cody-Waite split of 2*pi (C1 has 12 mantissa bits -> n*C1 exact for n<2^11)
C1 = float(np.float32(np.floor(2 * np.pi * 2**9) / 2**9))
C2 = float(np.float32(2 * np.pi - C1))
C3 = float(np.float32(2 * np.pi - C1 - float(np.float32(2 * np.pi - C1))))
HALF_PI = float(np.pi / 2)

# qk output-feature permutation: 12 slices of 128 rows
#   slices 0..7  : [q_h dims 0:64 | k_h dims 0:64]   (rope rows)
#   slice  8, 9  : q tails (dims 64:96) of heads 0..3 / 4..7
#   slice 10,11  : k tails of heads 0..3 / 4..7
def _qk_colperm():
    cols = []
    for h in range(H):
        cols += list(range(96 * h, 96 * h + 64))          # q_h 0:64
        cols += list(range(768 + 96 * h, 768 + 96 * h + 64))  # k_h 0:64
    for h in range(H):
        cols += list(range(96 * h + 64, 96 * h + 96))     # q tails
    for h in range(H):
        cols += list(range(768 + 96 * h + 64, 768 + 96 * h + 96))  # k tails
    return np.array(cols)

QK_PERM = _qk_colperm()

# merged slab layout: per head h (288 rows x TPC cols, bf16):
#   rows 288h+  0.. 96 : q head h, feature-major (rope dims 0:64, tail 64:96)
#   rows 288h+ 96..192 : k head h, feature-major
#   rows 288h+192..288 : v head h, token-major packed (4 x (128tok x 96) flat)
SLABR = 288 * H  # 2304

_prog_cache = {}


def build_program():
    key = 0
    if key in _prog_cache:
        return _prog_cache[key]
    nc = bacc.Bacc("TRN2", target_bir_lowering=False, debug=False,
                   num_devices=N_CORES)
    alu = mybir.AluOpType
    act = mybir.ActivationFunctionType

    # ---------------- DRAM I/O ----------------
    xT_d = nc.dram_tensor("xT", [D, TPC], F32R, kind="ExternalInput")
    wqk_d = nc.dram_tensor("Wqk", [128, KD * 1536], BF16,
                           kind="ExternalInput")
    wv_d = nc.dram_tensor("Wv", [128, KD * D], BF16, kind="ExternalInput")
    teq_d = nc.dram_tensor("te_q", [NTYPE, 1536], BF16, kind="ExternalInput")
    tek_d = nc.dram_tensor("te_k", [NTYPE, 1536], BF16, kind="ExternalInput")
    bv_d = nc.dram_tensor("bvrow", [1, D], BF16, kind="ExternalInput")
    qtype_d = nc.dram_tensor("qtype", [1, TPC], F32R, kind="ExternalInput")
    ktype_d = nc.dram_tensor("ktype", [1, TPC], F32R, kind="ExternalInput")
    pos4_d = nc.dram_tensor("pos4", [4, TPC], F32R, kind="ExternalInput")
    invf_d = nc.dram_tensor("invf", [128, 1], F32, kind="ExternalInput")
    w1_d = nc.dram_tensor("W1", [128, KD * 2 * DFF], BF16,
                          kind="ExternalInput")
    w2_d = nc.dram_tensor("W2", [128, 16 * D], BF16, kind="ExternalInput")
    b1a_d = nc.dram_tensor("b1a", [128, 16], F32, kind="ExternalInput")
    b1g_d = nc.dram_tensor("b1g", [128, 16], F32, kind="ExternalInput")
    bf2_d = nc.dram_tensor("bf2", [128, KD], F32, kind="ExternalInput")
    masks_d = nc.dram_tensor("masks", [128, 4 * 512], BF16,
                             kind="ExternalInput")
    r128_d = nc.dram_tensor("R128", [128, 128], F32R, kind="ExternalInput")
    onesrowb_d = nc.dram_tensor("onesrowb", [1, 128], BF16,
                                kind="ExternalInput")
    b4_d = nc.dram_tensor("B4", [4, 128], F32R, kind="ExternalInput")
    ones_d = nc.dram_tensor("ones128", [1, 128], F32R, kind="ExternalInput")
    onescol_d = nc.dram_tensor("onescol", [128, 1], F32R, kind="ExternalInput")
    onescolb_d = nc.dram_tensor("onescolb", [128, 1], BF16,
                                kind="ExternalInput")
    iota_d = nc.dram_tensor("iota16", [16, 1], F32, kind="ExternalInput")
    eps_d = nc.dram_tensor("epsc", [1, 1], F32, kind="ExternalInput")
    outT_d = nc.dram_tensor("outT", [D, TPC], F32, kind="ExternalOutput")

    with tile.TileContext(nc) as tc:
        with tc.tile_pool(name="glob", bufs=1) as glob, \
             tc.tile_pool(name="dram", bufs=1, space="DRAM") as dram:
            # exchange slabs
            slab_in = dram.tile([SLABR, TPC], BF16, tag="slab_in")
            slab_out = dram.tile([SLABR, TPC], BF16, tag="slab_out")
            slab2_in = dram.tile([D, TPC], BF16, tag="slab2_in")
            slab2_out = dram.tile([D, TPC], BF16, tag="slab2_out")
            # ---- persistent constants / activations ----
            ones_sb = glob.tile([1, 128], F32R, tag="ones")
            nc.sync.dma_start(out=ones_sb[:], in_=ones_d[:])
            onescol_sb = glob.tile([128, 1], F32R, tag="onescol")
            nc.sync.dma_start(out=onescol_sb[:], in_=onescol_d[:])
            onescolb_sb = glob.tile([128, 1], BF16, tag="onescolb")
            nc.sync.dma_start(out=onescolb_sb[:], in_=onescolb_d[:])
            onesrowb_sb = glob.tile([1, 128], BF16, tag="onesrowb")
            nc.sync.dma_start(out=onesrowb_sb[:], in_=onesrowb_d[:])
            iota_sb = glob.tile([16, 1], F32, tag="iota")
            nc.sync.dma_start(out=iota_sb[:], in_=iota_d[:])
            eps_sb = glob.tile([1, 1], F32, tag="eps")
            nc.sync.dma_start(out=eps_sb[:], in_=eps_d[:])
            xT = []
            for k in range(KD):
                t = glob.tile([128, TPC], F32R, tag=f"xT{k}")
                nc.sync.dma_start(out=t[:], in_=xT_d[128 * k:128 * (k + 1), :])
                xT.append(t)

            def layernorm_stats(pool, src_tiles, tag):
                """src (fp32r aps, 6 x (128,TPC)) -> (alpha_row, beta_row)."""
                ctx = contextlib.ExitStack()
                ps_pool = ctx.enter_context(
                    tc.tile_pool(name=f"{tag}ps", bufs=1, space="PSUM"))
                sums = ps_pool.tile([1, TPC], F32, tag=f"{tag}sums")
                sumsq = ps_pool.tile([1, TPC], F32, tag=f"{tag}sumsq")
                for k in range(KD):
                    sq = pool.tile([128, TPC], F32R, tag=f"{tag}sq")
                    nc.scalar.activation(out=sq[:],
                                         in_=src_tiles[k].bitcast(F32),
                                         func=act.Square)
                    nc.tensor.matmul(sums[:], onescol_sb[:], src_tiles[k],
                                     start=(k == 0), stop=(k == KD - 1))
                    nc.tensor.matmul(sumsq[:], onescol_sb[:], sq[:],
                                     start=(k == 0), stop=(k == KD - 1))
                mean = pool.tile([1, TPC], F32, tag=f"{tag}mean")
                nc.vector.tensor_scalar(out=mean[:], in0=sums[:],
                                        scalar1=1.0 / D, scalar2=None,
                                        op0=alu.mult)
                m2 = pool.tile([1, TPC], F32, tag=f"{tag}m2")
                nc.vector.tensor_tensor(out=m2[:], in0=mean[:], in1=mean[:],
                                        op=alu.mult)
                var = pool.tile([1, TPC], F32, tag=f"{tag}var")
                nc.vector.scalar_tensor_tensor(
                    out=var[:], in0=sumsq[:], scalar=1.0 / D, in1=m2[:],
                    op0=alu.mult, op1=alu.subtract)
                std = pool.tile([1, TPC], F32, tag=f"{tag}std")
                nc.scalar.activation(out=std[:], in_=var[:], func=act.Sqrt,
                                     bias=eps_sb[:])
                alpha = pool.tile([1, TPC], F32R, tag=f"{tag}alpha")
                with nc.allow_low_precision(reason="fp32r bcast rhs"):
                    nc.vector.reciprocal(out=alpha[:], in_=std[:])
                beta = pool.tile([1, TPC], F32R, tag=f"{tag}beta")
                nc.vector.scalar_tensor_tensor(
                    out=beta[:], in0=mean[:], scalar=-1.0,
                    in1=alpha[:].bitcast(F32), op0=alu.mult, op1=alu.mult)
                ctx.close()
                return alpha, beta

            def ln_apply(pool, scratch, src_f32_aps, alpha, beta, tag,
                         out_dt=F32R):
                """out[k] = src*alpha_bcast + beta_bcast (g/b folded into W)."""
                ctx = contextlib.ExitStack()
                ps_pool = ctx.enter_context(
                    tc.tile_pool(name=f"{tag}ps", bufs=1, space="PSUM"))
                ab = ps_pool.tile([128, TPC], F32, tag=f"{tag}ab")
                nc.tensor.matmul(ab[:], ones_sb[:], alpha[:], start=True,
                                 stop=True)
                bb = ps_pool.tile([128, TPC], F32, tag=f"{tag}bb")
                nc.tensor.matmul(bb[:], ones_sb[:], beta[:], start=True,
                                 stop=True)
                ab_sb = pool.tile([128, TPC], F32, tag=f"{tag}absb")
                nc.vector.tensor_copy(out=ab_sb[:], in_=ab[:])
                bb_sb = pool.tile([128, TPC], F32, tag=f"{tag}bbsb")
                nc.vector.tensor_copy(out=bb_sb[:], in_=bb[:])
                ctx.close()
                outs = []
                for k in range(KD):
                    t1 = scratch.tile([128, TPC], F32, tag=f"{tag}t1")
                    nc.vector.tensor_tensor(out=t1[:], in0=src_f32_aps[k],
                                            in1=ab_sb[:], op=alu.mult)
                    o = pool.tile([128, TPC], out_dt, tag=f"{tag}o{k}")
                    nc.vector.tensor_tensor(out=o[:], in0=t1[:], in1=bb_sb[:],
                                            op=alu.add)
                    outs.append(o)
                return outs

            # ================= PHASE 1 =================
            with contextlib.ExitStack() as p1:
                p1w = p1.enter_context(tc.tile_pool(name="p1w", bufs=1))
                p1t = p1.enter_context(tc.tile_pool(name="p1t", bufs=2))
                p1qk = p1.enter_context(
                    tc.tile_pool(name="p1qk", bufs=2, space="PSUM"))

                wqk_sb = p1w.tile([128, KD * 1536], BF16, tag="wqk")
                nc.sync.dma_start(out=wqk_sb[:], in_=wqk_d[:])
                wv_sb = p1w.tile([128, KD * D], BF16, tag="wv")
                nc.sync.dma_start(out=wv_sb[:], in_=wv_d[:])
                teq_sb = p1w.tile([NTYPE, 1536], BF16, tag="teq")
                nc.sync.dma_start(out=teq_sb[:], in_=teq_d[:])
                tek_sb = p1w.tile([NTYPE, 1536], BF16, tag="tek")
                nc.sync.dma_start(out=tek_sb[:], in_=tek_d[:])
                bv_sb = p1w.tile([1, D], BF16, tag="bv")
                nc.sync.dma_start(out=bv_sb[:], in_=bv_d[:])
                r128_sb = p1w.tile([128, 128], F32R, tag="r128")
                nc.sync.dma_start(out=r128_sb[:], in_=r128_d[:])
                b4_sb = p1w.tile([4, 128], F32R, tag="b4")
                nc.sync.dma_start(out=b4_sb[:], in_=b4_d[:])
                invf_sb = p1w.tile([128, 1], F32, tag="invf")
                nc.sync.dma_start(out=invf_sb[:], in_=invf_d[:])
                pos4_sb = p1w.tile([4, TPC], F32R, tag="pos4")
                nc.sync.dma_start(out=pos4_sb[:], in_=pos4_d[:])
                qt_sb = p1w.tile([1, TPC], F32R, tag="qt")
                nc.sync.dma_start(out=qt_sb[:], in_=qtype_d[:])
                kt_sb = p1w.tile([1, TPC], F32R, tag="kt")
                nc.sync.dma_start(out=kt_sb[:], in_=ktype_d[:])

                # LN1
                a1, be1 = layernorm_stats(p1t, [x[:] for x in xT], "l1")
                xn = ln_apply(p1w, p1t, [x[:].bitcast(F32) for x in xT],
                              a1, be1, "l1a", out_dt=BF16)

                # one-hot type codes (16, TPC)
                p1misc = p1.enter_context(
                    tc.tile_pool(name="p1misc", bufs=1, space="PSUM"))

                def onehot(row_sb, tag):
                    bc = p1misc.tile([16, TPC], F32, tag="ohbc")
                    nc.tensor.matmul(bc[:], ones_sb[:, 0:16], row_sb[:],
                                     start=True, stop=True)
                    oh = p1w.tile([16, TPC], BF16, tag=f"{tag}oh")
                    nc.vector.tensor_scalar(out=oh[:], in0=bc[:],
                                            scalar1=iota_sb[:], scalar2=None,
                                            op0=alu.is_equal)
                    return oh
                oh_q = onehot(qt_sb, "q")
                oh_k = onehot(kt_sb, "k")

                # cos/sin tiles (128, TPC): rows 0:64 q-axes, 64:128 k-axes
                pm = p1misc.tile([128, TPC], F32, tag="pm")
                nc.tensor.matmul(pm[:], b4_sb[:], pos4_sb[:], start=True,
                                 stop=True)
                f_t = p1t.tile([128, TPC], F32, tag="f")
                nc.vector.tensor_scalar(out=f_t[:], in0=pm[:],
                                        scalar1=invf_sb[:], scalar2=None,
                                        op0=alu.mult)
                nt = p1t.tile([128, TPC], F32, tag="nt")
                nc.vector.tensor_scalar(out=nt[:], in0=f_t[:],
                                        scalar1=float(1.0 / (2 * np.pi)),
                                        scalar2=None, op0=alu.mult)
                n_i = p1t.tile([128, TPC], I32, tag="ni")
                nc.vector.tensor_copy(out=n_i[:], in_=nt[:])
                n_f = p1t.tile([128, TPC], F32, tag="nf")
                nc.vector.tensor_copy(out=n_f[:], in_=n_i[:])
                fr = p1t.tile([128, TPC], F32, tag="fr")
                nc.vector.scalar_tensor_tensor(out=fr[:], in0=n_f[:],
                                               scalar=-C1, in1=f_t[:],
                                               op0=alu.mult, op1=alu.add)
                nc.vector.scalar_tensor_tensor(out=fr[:], in0=n_f[:],
                                               scalar=-C2, in1=fr[:],
                                               op0=alu.mult, op1=alu.add)
                nc.vector.scalar_tensor_tensor(out=fr[:], in0=n_f[:],
                                               scalar=-C3, in1=fr[:],
                                               op0=alu.mult, op1=alu.add)
                s_t = p1w.tile([128, TPC], F32, tag="sin")
                nc.scalar.activation(out=s_t[:], in_=fr[:], func=act.Sin)
                af = p1t.tile([128, TPC], F32, tag="af")
                nc.scalar.activation(out=af[:], in_=fr[:], func=act.Abs)
                ca = p1t.tile([128, TPC], F32, tag="ca")
                nc.vector.tensor_scalar(out=ca[:], in0=af[:], scalar1=-1.0,
                                        scalar2=HALF_PI, op0=alu.mult,
                                        op1=alu.add)
                c_t = p1w.tile([128, TPC], F32, tag="cos")
                nc.scalar.activation(out=c_t[:], in_=ca[:], func=act.Sin)
                # fold score scale 1/sqrt(HD) into q: scale c,s rows 0:64
                nc.vector.tensor_scalar(out=c_t[0:64, :], in0=c_t[0:64, :],
                                        scalar1=float(SCALE), scalar2=None,
                                        op0=alu.mult)
                nc.vector.tensor_scalar(out=s_t[0:64, :], in0=s_t[0:64, :],
                                        scalar1=float(SCALE), scalar2=None,
                                        op0=alu.mult)

                # qk slices: matmul + type emb, then rope / tails -> slab
                for s in range(12):
                    qk_ps = p1qk.tile([128, TPC], F32, tag="qkps")
                    for k in range(KD):
                        nc.tensor.matmul(
                            qk_ps[:],
                            wqk_sb[:, 1536 * k + 128 * s:1536 * k + 128 * (s + 1)],
                            xn[k][:], start=(k == 0), stop=False)
                    nc.tensor.matmul(qk_ps[:],
                                     teq_sb[:, 128 * s:128 * (s + 1)],
                                     oh_q[:], start=False, stop=False)
                    nc.tensor.matmul(qk_ps[:],
                                     tek_sb[:, 128 * s:128 * (s + 1)],
                                     oh_k[:], start=False, stop=True)
                    if s < 8:
                        # rope: q_h 0:64 | k_h 0:64
                        rsb = p1t.tile([128, TPC], F32R, tag="rsb")
                        nc.vector.tensor_copy(out=rsb[:], in_=qk_ps[:])
                        rot = p1qk.tile([128, TPC], F32, tag="rot")
                        nc.tensor.matmul(rot[:], r128_sb[:], rsb[:],
                                         start=True, stop=True)
                        t1 = p1t.tile([128, TPC], F32, tag="rt1")
                        nc.vector.tensor_tensor(out=t1[:],
                                                in0=rsb[:].bitcast(F32),
                                                in1=c_t[:], op=alu.mult)
                        t2 = p1t.tile([128, TPC], F32, tag="rt2")
                        nc.vector.tensor_tensor(out=t2[:], in0=rot[:],
                                                in1=s_t[:], op=alu.mult)
                        qkr = p1t.tile([128, TPC], BF16, tag="qkr")
                        nc.vector.tensor_tensor(out=qkr[:], in0=t1[:],
                                                in1=t2[:], op=alu.add)
                        h = s
                        nc.scalar.dma_start(
                            out=slab_in[288 * h + 0:288 * h + 64, :],
                            in_=qkr[0:64, :])
                        nc.scalar.dma_start(
                            out=slab_in[288 * h + 96:288 * h + 160, :],
                            in_=qkr[64:128, :])
                    else:
                        # tails: s=8,9 q tails h0..3/h4..7 (scale by 1/sqrt(HD))
                        # s=10,11 k tails
                        tl = p1t.tile([128, TPC], BF16, tag="tail")
                        sc = float(SCALE) if s < 10 else 1.0
                        nc.vector.tensor_scalar(out=tl[:], in0=qk_ps[:],
                                                scalar1=sc, scalar2=None,
                                                op0=alu.mult)
                        base = 64 if s < 10 else 160  # q tail at +64, k at +160
                        for j in range(4):
                            h = 4 * (s % 2) + j
                            nc.scalar.dma_start(
                                out=slab_in[288 * h + base:288 * h + base + 32, :],
                                in_=tl[32 * j:32 * (j + 1), :])

                # v (token-major): 4 tok-slices x 2 halves of 384 cols
                for ts_ in range(4):
                    for hf in range(2):
                        v_ps = p1qk.tile([128, 384], F32, tag="vps")
                        for k in range(KD):
                            nc.tensor.matmul(
                                v_ps[:],
                                xn[k][:, 128 * ts_:128 * (ts_ + 1)],
                                wv_sb[:, D * k + 384 * hf:D * k + 384 * (hf + 1)],
                                start=(k == 0), stop=False)
                        nc.tensor.matmul(
                            v_ps[:], onesrowb_sb[:],
                            bv_sb[:, 384 * hf:384 * (hf + 1)],
                            start=False, stop=True)
                        v_sb1 = p1t.tile([128, 384], BF16, tag="vsb1")
                        nc.vector.tensor_copy(out=v_sb1[:], in_=v_ps[:])
                        # one batched DMA: (tok, head j, dim) -> v region of
                        # blocks h=4*hf+j at flat (288h+192)*TPC + 128*ts_*96
                        dst = bass.AP(
                            tensor=slab_in[:].tensor,
                            offset=(288 * 4 * hf + 192) * TPC + 128 * ts_ * 96,
                            ap=[[96, 128], [288 * TPC, 4], [1, 96]])
                        nc.sync.dma_start(out=dst, in_=v_sb1[:])

                nc.gpsimd.collective_compute(
                    "AllToAll", mybir.AluOpType.bypass,
                    replica_groups=[list(range(N_CORES))],
                    ins=[slab_in[:].opt()],
                    outs=[slab_out[:].opt()])

            # ================= PHASE 2 =================
            # pool for FFN weights: spans phases 2+3 only
            wff_ctx = contextlib.ExitStack()
            wff = wff_ctx.enter_context(tc.tile_pool(name="wff", bufs=1))
            with contextlib.ExitStack() as p2:
                p2w = p2.enter_context(tc.tile_pool(name="p2w", bufs=1))
                p2t = p2.enter_context(tc.tile_pool(name="p2t", bufs=3))
                p2ps = p2.enter_context(
                    tc.tile_pool(name="p2ps", bufs=4, space="PSUM"))
                p2o = p2.enter_context(
                    tc.tile_pool(name="p2o", bufs=2, space="PSUM"))
                p2rb = p2.enter_context(
                    tc.tile_pool(name="p2rb", bufs=2, space="PSUM"))

                masks_sb = p2w.tile([128, 4 * 512], BF16, tag="masks")
                nc.sync.dma_start(out=masks_sb[:], in_=masks_d[:])

                # FFN weights + biases: prefetched during phase 2 (after the
                # qkv collective is enqueued, keeping its DMA rings quiet)
                w1_sb = wff.tile([128, KD * 2 * DFF], BF16, tag="w1")
                nc.scalar.dma_start(out=w1_sb[:], in_=w1_d[:])
                w2_sb = wff.tile([128, 16 * D], BF16, tag="w2")
                nc.scalar.dma_start(out=w2_sb[:], in_=w2_d[:])
                b1a_sb = wff.tile([128, 16], F32, tag="b1a")
                nc.scalar.dma_start(out=b1a_sb[:], in_=b1a_d[:])
                b1g_sb = wff.tile([128, 16], F32, tag="b1g")
                nc.scalar.dma_start(out=b1g_sb[:], in_=b1g_d[:])
                bf2_sb = wff.tile([128, KD], F32, tag="bf2")
                nc.scalar.dma_start(out=bf2_sb[:], in_=bf2_d[:])

                for bb_ in range(2):
                    qT = p2w.tile([96, 2048], BF16, tag=f"qT{bb_}")
                    kT = p2w.tile([96, 2048], BF16, tag=f"kT{bb_}")
                    v_sb = p2w.tile([128, 16, 97], BF16, tag=f"v{bb_}")
                    ones_bc = bass.AP(
                        tensor=onescolb_sb[:].tensor,
                        offset=onescolb_sb[:].offset,
                        ap=[[1, 128], [0, 16], [0, 1]])
                    nc.sync.dma_start(out=v_sb[:, :, 96:97], in_=ones_bc)
                    for u in range(4):
                        blk = 288 * (4 * bb_ + u)
                        nc.sync.dma_start(
                            out=qT[:, 512 * u:512 * (u + 1)],
                            in_=slab_out[blk + 0:blk + 96, :])
                        nc.sync.dma_start(
                            out=kT[:, 512 * u:512 * (u + 1)],
                            in_=slab_out[blk + 96:blk + 192, :])
                    # v: one gather DMA per source core (tok, tok-slice, dim)
                    vfull = v_sb[:]
                    for u in range(4):
                        nc.sync.dma_start(
                            out=bass.AP(tensor=vfull.tensor,
                                        offset=vfull.offset + 97 * 4 * u,
                                        ap=[list(vfull.ap[0]), [97, 4],
                                            [1, 96]]),
                            in_=bass.AP(
                                tensor=slab_out[:].tensor,
                                offset=(288 * (4 * bb_ + u) + 192) * TPC,
                                ap=[[96, 128], [128 * 96, 4], [1, 96]]))

                    for Q in reversed(range(NSUP)):
                        o_ps = p2o.tile([97, 512], F32, tag="ops", name="ops")
                        nkt = 4 * Q + 4
                        for kt in range(nkt):
                            s_ps = p2ps.tile([128, 512], F32, tag="sps",
                                             name="sps")
                            nc.tensor.matmul(
                                s_ps[:], kT[:, 128 * kt:128 * (kt + 1)],
                                qT[:, 512 * Q:512 * (Q + 1)],
                                start=True, stop=True)
                            e_sb = p2t.tile([128, 512], BF16, tag="esb",
                                            name="esb")
                            nc.scalar.activation(out=e_sb[:], in_=s_ps[:],
                                                 func=act.Exp)
                            dj = kt - 4 * Q
                            if dj >= 0:
                                nc.vector.tensor_tensor(
                                    out=e_sb[:], in0=e_sb[:],
                                    in1=masks_sb[:, 512 * dj:512 * (dj + 1)],
                                    op=alu.mult)
                            nc.tensor.matmul(o_ps[:], v_sb[:, kt, :], e_sb[:],
                                             start=(kt == 0),
                                             stop=(kt == nkt - 1))
                        # per-unit softmax normalization, pipelined behind
                        # the next unit's matmuls
                        j = 4 * bb_ + Q
                        rec = p2t.tile([1, 512], F32R, tag="rec")
                        with nc.allow_low_precision(reason="softmax denom"):
                            nc.vector.reciprocal(out=rec[:],
                                                 in_=o_ps[96:97, :])
                        rb_ps = p2rb.tile([96, 512], F32, tag="rb")
                        nc.tensor.matmul(rb_ps[:], ones_sb[:, 0:96], rec[:],
                                         start=True, stop=True)
                        rb_sb = p2t.tile([96, 512], F32, tag="rbsb")
                        nc.vector.tensor_copy(out=rb_sb[:], in_=rb_ps[:])
                        onrm = p2t.tile([96, 512], BF16, tag="onrm")
                        nc.vector.tensor_tensor(out=onrm[:],
                                                in0=o_ps[0:96, :],
                                                in1=rb_sb[:], op=alu.mult)
                        nc.scalar.dma_start(
                            out=slab2_in[96 * j:96 * (j + 1), :], in_=onrm[:])

            nc.gpsimd.collective_compute(
                "AllToAll", mybir.AluOpType.bypass,
                replica_groups=[list(range(N_CORES))],
                ins=[slab2_in[:].opt()], outs=[slab2_out[:].opt()])

            # ================= PHASE 3 =================
            with contextlib.ExitStack() as p3:
                p3w = p3.enter_context(tc.tile_pool(name="p3w", bufs=1))
                p3t = p3.enter_context(tc.tile_pool(name="p3t", bufs=2))

                x2 = []
                for k in range(KD):
                    o_sb = p3t.tile([128, TPC], BF16, tag="osb")
                    nc.sync.dma_start(out=o_sb[:],
                                      in_=slab2_out[128 * k:128 * (k + 1), :])
                    t = p3w.tile([128, TPC], F32R, tag=f"x2_{k}")
                    nc.vector.tensor_tensor(out=t[:], in0=o_sb[:],
                                            in1=xT[k][:].bitcast(F32),
                                            op=alu.add)
                    x2.append(t)

                a2, be2 = layernorm_stats(p3t, [t[:] for t in x2], "l2")
                x2n = ln_apply(p3w, p3t, [t[:].bitcast(F32) for t in x2],
                               a2, be2, "l2a", out_dt=BF16)

                # fc1 from prefetched packed w1 tile
                a_tiles = []
                sw = []
                with tc.tile_pool(name="p3h", bufs=2, space="PSUM") as p3h:
                    for g in range(8):           # g<4: a-half, g>=4: gate-half
                        for mi in range(4):
                            i = 4 * (g % 4) + mi
                            col = 512 * g + 128 * mi
                            h_ps = p3h.tile([128, TPC], F32, tag="hps")
                            for k in range(KD):
                                nc.tensor.matmul(
                                    h_ps[:],
                                    w1_sb[:, 4096 * k + col:4096 * k + col + 128],
                                    x2n[k][:],
                                    start=(k == 0), stop=(k == KD - 1))
                            if g < 4:
                                a_sb = p3w.tile([128, TPC], BF16, tag=f"a{i}")
                                nc.vector.tensor_scalar(
                                    out=a_sb[:], in0=h_ps[:],
                                    scalar1=b1a_sb[:, i:i + 1],
                                    scalar2=None, op0=alu.add)
                                a_tiles.append(a_sb)
                            else:
                                sil = p3t.tile([128, TPC], BF16, tag="sil")
                                nc.scalar.activation(
                                    out=sil[:], in_=h_ps[:], func=act.Silu,
                                    bias=b1g_sb[:, i:i + 1])
                                swt = p3w.tile([128, TPC], BF16, tag=f"sw{i}")
                                nc.vector.tensor_tensor(
                                    out=swt[:], in0=sil[:],
                                    in1=a_tiles[i][:], op=alu.mult)
                                sw.append(swt)

                # fc2: k2-outer, 6 persistent ff psum banks, prefetched w2
                with tc.tile_pool(name="p3f", bufs=1, space="PSUM") as p3f:
                    ff_ps = [p3f.tile([128, TPC], F32, tag=f"ff{d}",
                                      name=f"ff{d}")
                             for d in range(KD)]
                    for k2 in range(16):
                        for d in range(KD):
                            nc.tensor.matmul(
                                ff_ps[d][:],
                                w2_sb[:, D * k2 + 128 * d:D * k2 + 128 * (d + 1)],
                                sw[k2][:],
                                start=(k2 == 0), stop=(k2 == 15))
                    for d in range(KD):
                        t = p3t.tile([128, TPC], F32, tag="fft")
                        nc.vector.tensor_scalar(out=t[:], in0=ff_ps[d][:],
                                                scalar1=bf2_sb[:, d:d + 1],
                                                scalar2=None, op0=alu.add)
                        o = p3t.tile([128, TPC], F32, tag="oout")
                        nc.vector.tensor_tensor(out=o[:], in0=t[:],
                                                in1=x2[d][:].bitcast(F32),
                                                op=alu.add)
                        nc.sync.dma_start(
                            out=outT_d[128 * d:128 * (d + 1), :], in_=o[:])
            wff_ctx.close()

    nc.compile()
    _prog_cache[key] = nc
    return nc


def _host_inputs(x_type, x_value, seq_order, W_attn, type_emb, g1, b1, g2, b2,
                 W_fc1, b_fc1, W_fc2, b_fc2):
    f32 = np.float32
    x_type = np.asarray(x_type)
    seq_order = np.asarray(seq_order)
    x_value = np.asarray(x_value, dtype=f32)
    W_attn = np.asarray(W_attn, dtype=f32)
    type_emb = np.asarray(type_emb, dtype=f32)
    W_fc1 = np.asarray(W_fc1, dtype=f32)
    W_fc2 = np.asarray(W_fc2, dtype=f32)
    g1 = np.asarray(g1, f32); b1 = np.asarray(b1, f32)
    g2 = np.asarray(g2, f32); b2 = np.asarray(b2, f32)
    b_fc1 = np.asarray(b_fc1, f32); b_fc2 = np.asarray(b_fc2, f32)

    # fold LN1 gain into W_attn, LN1 bias row into type emb / v bias row
    Wqk_s = g1[:, None] * W_attn[:, :1536]
    Wv_s = g1[:, None] * W_attn[:, 1536:]
    bqk_row = b1 @ W_attn[:, :1536]          # (1536,)
    bv_row = (b1 @ W_attn[:, 1536:]).reshape(1, D)

    def pack_k(w):
        # (KD*128, C) -> (128, KD*C): chunk k at cols [C*k : C*(k+1)]
        c = w.shape[1]
        return np.ascontiguousarray(
            w.reshape(KD, 128, c).transpose(1, 0, 2).reshape(128, KD * c))

    wqk_full = pack_k(Wqk_s[:, QK_PERM]).astype(BF16NP)
    te_full = type_emb[:, QK_PERM] + bqk_row[QK_PERM][None, :]
    q_origin = QK_PERM < 768
    te_q = np.where(q_origin[None, :], te_full, 0.0).astype(BF16NP)
    te_k = np.where(~q_origin[None, :], te_full, 0.0).astype(BF16NP)

    invf16 = (1.0 / THETA ** (np.arange(0, DR, 2, dtype=f32) / DR)).astype(f32)
    invf_col = invf16[(np.arange(128) % 32) // 2].reshape(128, 1)

    # masks: block (128k x 512q), mask[kk, qq] = 1 if qq >= kk + 128*dj
    kk = np.arange(128)[:, None]
    qq = np.arange(512)[None, :]
    masks = np.concatenate(
        [(qq >= kk + 128 * dj).astype(f32) for dj in range(4)],
        axis=1).astype(BF16NP)

    # rot lhsT: lhsT[k, m] = P[m, k];  P[2i, 2i+1] = -1, P[2i+1, 2i] = +1
    R = np.zeros((128, 128), f32)
    for i in range(64):
        R[2 * i + 1, 2 * i] = -1.0
        R[2 * i, 2 * i + 1] = 1.0
    B4m = np.zeros((4, 128), f32)
    B4m[0, 0:32] = 1.0; B4m[1, 32:64] = 1.0
    B4m[2, 64:96] = 1.0; B4m[3, 96:128] = 1.0

    # fold LN2 gain/bias into W_fc1 / its bias
    W1_s = (g2[:, None] * W_fc1).astype(BF16NP)
    b_fc1_eff = b_fc1 + b2 @ W_fc1

    # W2 packed over its 16 contraction chunks: (2048, 768) -> (128, 16*768)
    W2_p = np.ascontiguousarray(
        W_fc2.reshape(16, 128, D).transpose(1, 0, 2).reshape(128, 16 * D))

    common = {
        "Wqk": wqk_full, "Wv": pack_k(Wv_s).astype(BF16NP),
        "te_q": te_q, "te_k": te_k, "bvrow": bv_row.astype(BF16NP),
        "invf": invf_col,
        "W1": pack_k(W1_s), "W2": W2_p.astype(BF16NP),
        "b1a": b_fc1_eff[:2048].reshape(16, 128).T.copy(),
        "b1g": b_fc1_eff[2048:].reshape(16, 128).T.copy(),
        "bf2": b_fc2.reshape(6, 128).T.copy(),
        "masks": masks, "R128": R, "B4": B4m,
        "ones128": np.ones((1, 128), f32),
        "onescol": np.ones((128, 1), f32),
        "onescolb": np.ones((128, 1), BF16NP),
        "onesrowb": np.ones((1, 128), BF16NP),
        "iota16": np.arange(16, dtype=f32).reshape(16, 1),
        "epsc": np.full((1, 1), EPS, f32),
    }
    in_maps = []
    for c in range(N_CORES):
        b = c // 4
        t0 = 512 * (c % 4)
        m = dict(common)
        m["xT"] = np.ascontiguousarray(x_value[b, t0:t0 + TPC, :].T)
        m["qtype"] = x_type[b, t0:t0 + TPC].astype(f32).reshape(1, TPC)
        m["ktype"] = x_type[b, t0 + 1:t0 + TPC + 1].astype(f32).reshape(1, TPC)
        pos4 = np.stack([
            seq_order[0, b, t0:t0 + TPC],
            seq_order[1, b, t0:t0 + TPC],
            seq_order[0, b, t0 + 1:t0 + TPC + 1],
            seq_order[1, b, t0 + 1:t0 + TPC + 1],
        ]).astype(f32)
        m["pos4"] = pos4
        in_maps.append(m)
    return in_maps


def kernel(**inputs):
    nc = build_program()
    in_maps = _host_inputs(**inputs)
    res = run_bass_kernel_spmd(nc, in_maps, list(range(N_CORES)), trace=False)
    out = np.empty((B, T, D), np.float32)
    for c in range(N_CORES):
        b = c // 4
        t0 = 512 * (c % 4)
        out[b, t0:t0 + TPC, :] = res.results[c]["outT"].T
    return out


# revision 37
# speedup vs baseline: 1.0078x; 1.0078x over previous
"""Trainium2 Bass kernel for nn_DecoderLayer_90967407329666.

Decoder layer: LN1 -> QKV (+type emb) -> multi-axis RoPE -> causal SDPA
-> residual -> LN2 -> SwiGLU FFN -> residual.  B=2, T=2048, D=768, H=8,
DFF=2048, NTYPE=16, NAX=2 rotary axes of 32 dims each.

Sharding (8 cores):
  Phase 1 (token-parallel): each core owns 512 tokens (core c: batch c//4,
    tokens 512*(c%4)...) and computes LN1 + q,k (feature-major) + v
    (token-major) + type-emb + RoPE for those tokens, all 8 heads.
  Single AllToAll: block h carries head h's q,k,v (288 rows bf16) ->
    core c ends up with head c for ALL 4096 tokens.
  Phase 2 (head-parallel): core c runs full causal attention for head c,
    both batches; per-unit softmax normalization pipelined behind matmuls.
  AllToAll #2 (bf16): o goes back token-parallel.
  Phase 3 (token-parallel): residual + LN2 + SwiGLU FFN + residual.

dtypes: weights + exchange slabs in bf16; residual stream, LN stats and
position/angle math in fp32/fp32r (fp32r matmuls run at full PE rate for
free-dim >= 256, same as bf16).  LN gains/biases are folded into the
weights host-side (W' = g*W, bias row b@W folded into type-emb / biases).
A tiny AllToAll issued at t=0 absorbs the cross-core rendezvous skew.
"""

import sys

sys.path.insert(0, "/opt/trn_rl_repo")

import numpy as np
import ml_dtypes

import contextlib

import concourse.bacc as bacc
import concourse.bass as bass
import concourse.tile as tile
from concourse import mybir
from concourse.bass_utils import run_bass_kernel_spmd

BF16NP = np.dtype(ml_dtypes.bfloat16)

# ---- problem constants (hardcoded per contest rules) ----
B, T = 2, 2048
D, H, DFF, NTYPE = 768, 8, 2048, 16
NAX = 2
HD = D // H            # 96
DR = HD // (NAX + 1)   # 32
EPS = 1e-5
THETA = 10000.0
N_CORES = 8
TPC = 512              # tokens per core
NSUP = 4               # supertiles per batch (2048/512)
KD = D // 128          # 6 contraction chunks over D
SCALE = 1.0 / np.sqrt(np.float32(HD))

F32 = mybir.dt.float32
F32R = mybir.dt.float32r
BF16 = mybir.dt.bfloat16
I32 = mybir.dt.int32

# Cody-Waite split of 2*pi (C1 has 12 mantissa bits -> n*C1 exact for n<2^11)
C1 = float(np.float32(np.floor(2 * np.pi * 2**9) / 2**9))
C2 = float(np.float32(2 * np.pi - C1))
C3 = float(np.float32(2 * np.pi - C1 - float(np.float32(2 * np.pi - C1))))
HALF_PI = float(np.pi / 2)

# qk output-feature permutation: 12 slices of 128 rows
#   slices 0..7  : [q_h dims 0:64 | k_h dims 0:64]   (rope rows)
#   slice  8, 9  : q tails (dims 64:96) of heads 0..3 / 4..7
#   slice 10,11  : k tails of heads 0..3 / 4..7
def _qk_colperm():
    cols = []
    for h in range(H):
        cols += list(range(96 * h, 96 * h + 64))          # q_h 0:64
        cols += list(range(768 + 96 * h, 768 + 96 * h + 64))  # k_h 0:64
    for h in range(H):
        cols += list(range(96 * h + 64, 96 * h + 96))     # q tails
    for h in range(H):
        cols += list(range(768 + 96 * h + 64, 768 + 96 * h + 96))  # k tails
    return np.array(cols)

QK_PERM = _qk_colperm()

# merged slab layout: per head h (288 rows x TPC cols, bf16):
#   rows 288h+  0.. 96 : q head h, feature-major (rope dims 0:64, tail 64:96)
#   rows 288h+ 96..192 : k head h, feature-major
#   rows 288h+192..288 : v head h, token-major packed (4 x (128tok x 96) flat)
SLABR = 288 * H  # 2304

_prog_cache = {}


def build_program():
    key = 0
    if key in _prog_cache:
        return _prog_cache[key]
    nc = bacc.Bacc("TRN2", target_bir_lowering=False, debug=False,
                   num_devices=N_CORES)
    alu = mybir.AluOpType
    act = mybir.ActivationFunctionType

    # ---------------- DRAM I/O ----------------
    xT_d = nc.dram_tensor("xT", [D, TPC], F32R, kind="ExternalInput")
    wqk_d = nc.dram_tensor("Wqk", [128, KD * 1536], BF16,
                           kind="ExternalInput")
    wv_d = nc.dram_tensor("Wv", [128, KD * D], BF16, kind="ExternalInput")
    teq_d = nc.dram_tensor("te_q", [NTYPE, 1536], BF16, kind="ExternalInput")
    tek_d = nc.dram_tensor("te_k", [NTYPE, 1536], BF16, kind="ExternalInput")
    bv_d = nc.dram_tensor("bvrow", [1, D], BF16, kind="ExternalInput")
    qtype_d = nc.dram_tensor("qtype", [1, TPC], F32R, kind="ExternalInput")
    ktype_d = nc.dram_tensor("ktype", [1, TPC], F32R, kind="ExternalInput")
    pos4_d = nc.dram_tensor("pos4", [4, TPC], F32R, kind="ExternalInput")
    invf_d = nc.dram_tensor("invf", [128, 1], F32, kind="ExternalInput")
    w1_d = nc.dram_tensor("W1", [128, KD * 2 * DFF], BF16,
                          kind="ExternalInput")
    w2_d = nc.dram_tensor("W2", [128, 16 * D], BF16, kind="ExternalInput")
    b1a_d = nc.dram_tensor("b1a", [128, 16], F32, kind="ExternalInput")
    b1g_d = nc.dram_tensor("b1g", [128, 16], F32, kind="ExternalInput")
    bf2_d = nc.dram_tensor("bf2", [128, KD], F32, kind="ExternalInput")
    masks_d = nc.dram_tensor("masks", [128, 4 * 512], BF16,
                             kind="ExternalInput")
    r128_d = nc.dram_tensor("R128", [128, 128], F32R, kind="ExternalInput")
    onesrowb_d = nc.dram_tensor("onesrowb", [1, 128], BF16,
                                kind="ExternalInput")
    b4_d = nc.dram_tensor("B4", [4, 128], F32R, kind="ExternalInput")
    ones_d = nc.dram_tensor("ones128", [1, 128], F32R, kind="ExternalInput")
    onescol_d = nc.dram_tensor("onescol", [128, 1], F32R, kind="ExternalInput")
    onescolb_d = nc.dram_tensor("onescolb", [128, 1], BF16,
                                kind="ExternalInput")
    iota_d = nc.dram_tensor("iota16", [16, 1], F32, kind="ExternalInput")
    eps_d = nc.dram_tensor("epsc", [1, 1], F32, kind="ExternalInput")
    outT_d = nc.dram_tensor("outT", [D, TPC], F32, kind="ExternalOutput")

    with tile.TileContext(nc) as tc:
        with tc.tile_pool(name="glob", bufs=1) as glob, \
             tc.tile_pool(name="dram", bufs=1, space="DRAM") as dram:
            # exchange slabs
            slab_in = dram.tile([SLABR, TPC], BF16, tag="slab_in")
            slab_out = dram.tile([SLABR, TPC], BF16, tag="slab_out")
            slab2_in = dram.tile([D, TPC], BF16, tag="slab2_in")
            slab2_out = dram.tile([D, TPC], BF16, tag="slab2_out")
            # ---- persistent constants / activations ----
            ones_sb = glob.tile([1, 128], F32R, tag="ones")
            nc.sync.dma_start(out=ones_sb[:], in_=ones_d[:])
            onescol_sb = glob.tile([128, 1], F32R, tag="onescol")
            nc.sync.dma_start(out=onescol_sb[:], in_=onescol_d[:])
            onescolb_sb = glob.tile([128, 1], BF16, tag="onescolb")
            nc.sync.dma_start(out=onescolb_sb[:], in_=onescolb_d[:])
            onesrowb_sb = glob.tile([1, 128], BF16, tag="onesrowb")
            nc.sync.dma_start(out=onesrowb_sb[:], in_=onesrowb_d[:])
            iota_sb = glob.tile([16, 1], F32, tag="iota")
            nc.sync.dma_start(out=iota_sb[:], in_=iota_d[:])
            eps_sb = glob.tile([1, 1], F32, tag="eps")
            nc.sync.dma_start(out=eps_sb[:], in_=eps_d[:])
            xT = []
            for k in range(KD):
                t = glob.tile([128, TPC], F32R, tag=f"xT{k}")
                nc.sync.dma_start(out=t[:], in_=xT_d[128 * k:128 * (k + 1), :])
                xT.append(t)

            def layernorm_stats(pool, src_tiles, tag):
                """src (fp32r aps, 6 x (128,TPC)) -> (alpha_row, beta_row)."""
                ctx = contextlib.ExitStack()
                ps_pool = ctx.enter_context(
                    tc.tile_pool(name=f"{tag}ps", bufs=1, space="PSUM"))
                sums = ps_pool.tile([1, TPC], F32, tag=f"{tag}sums")
                sumsq = ps_pool.tile([1, TPC], F32, tag=f"{tag}sumsq")
                for k in range(KD):
                    sq = pool.tile([128, TPC], F32R, tag=f"{tag}sq")
                    nc.scalar.activation(out=sq[:],
                                         in_=src_tiles[k].bitcast(F32),
                                         func=act.Square)
                    nc.tensor.matmul(sums[:], onescol_sb[:], src_tiles[k],
                                     start=(k == 0), stop=(k == KD - 1))
                    nc.tensor.matmul(sumsq[:], onescol_sb[:], sq[:],
                                     start=(k == 0), stop=(k == KD - 1))
                mean = pool.tile([1, TPC], F32, tag=f"{tag}mean")
                nc.vector.tensor_scalar(out=mean[:], in0=sums[:],
                                        scalar1=1.0 / D, scalar2=None,
                                        op0=alu.mult)
                m2 = pool.tile([1, TPC], F32, tag=f"{tag}m2")
                nc.vector.tensor_tensor(out=m2[:], in0=mean[:], in1=mean[:],
                                        op=alu.mult)
                var = pool.tile([1, TPC], F32, tag=f"{tag}var")
                nc.vector.scalar_tensor_tensor(
                    out=var[:], in0=sumsq[:], scalar=1.0 / D, in1=m2[:],
                    op0=alu.mult, op1=alu.subtract)
                std = pool.tile([1, TPC], F32, tag=f"{tag}std")
                nc.scalar.activation(out=std[:], in_=var[:], func=act.Sqrt,
                                     bias=eps_sb[:])
                alpha = pool.tile([1, TPC], F32R, tag=f"{tag}alpha")
                with nc.allow_low_precision(reason="fp32r bcast rhs"):
                    nc.vector.reciprocal(out=alpha[:], in_=std[:])
                beta = pool.tile([1, TPC], F32R, tag=f"{tag}beta")
                nc.vector.scalar_tensor_tensor(
                    out=beta[:], in0=mean[:], scalar=-1.0,
                    in1=alpha[:].bitcast(F32), op0=alu.mult, op1=alu.mult)
                ctx.close()
                return alpha, beta

            def ln_apply(pool, scratch, src_f32_aps, alpha, beta, tag,
                         out_dt=F32R):
                """out[k] = src*alpha_bcast + beta_bcast (g/b folded into W)."""
                ctx = contextlib.ExitStack()
                ps_pool = ctx.enter_context(
                    tc.tile_pool(name=f"{tag}ps", bufs=1, space="PSUM"))
                ab = ps_pool.tile([128, TPC], F32, tag=f"{tag}ab")
                nc.tensor.matmul(ab[:], ones_sb[:], alpha[:], start=True,
                                 stop=True)
                bb = ps_pool.tile([128, TPC], F32, tag=f"{tag}bb")
                nc.tensor.matmul(bb[:], ones_sb[:], beta[:], start=True,
                                 stop=True)
                ab_sb = pool.tile([128, TPC], F32, tag=f"{tag}absb")
                nc.vector.tensor_copy(out=ab_sb[:], in_=ab[:])
                bb_sb = pool.tile([128, TPC], F32, tag=f"{tag}bbsb")
                nc.vector.tensor_copy(out=bb_sb[:], in_=bb[:])
                ctx.close()
                outs = []
                for k in range(KD):
                    t1 = scratch.tile([128, TPC], F32, tag=f"{tag}t1")
                    nc.vector.tensor_tensor(out=t1[:], in0=src_f32_aps[k],
                                            in1=ab_sb[:], op=alu.mult)
                    o = pool.tile([128, TPC], out_dt, tag=f"{tag}o{k}")
                    nc.vector.tensor_tensor(out=o[:], in0=t1[:], in1=bb_sb[:],
                                            op=alu.add)
                    outs.append(o)
                return outs

            # ================= PHASE 1 =================
            with contextlib.ExitStack() as p1:
                p1w = p1.enter_context(tc.tile_pool(name="p1w", bufs=1))
                p1t = p1.enter_context(tc.tile_pool(name="p1t", bufs=2))
                p1qk = p1.enter_context(
                    tc.tile_pool(name="p1qk", bufs=2, space="PSUM"))

                wqk_sb = p1w.tile([128, KD * 1536], BF16, tag="wqk")
                nc.sync.dma_start(out=wqk_sb[:], in_=wqk_d[:])
                wv_sb = p1w.tile([128, KD * D], BF16, tag="wv")
                nc.sync.dma_start(out=wv_sb[:], in_=wv_d[:])
                teq_sb = p1w.tile([NTYPE, 1536], BF16, tag="teq")
                nc.sync.dma_start(out=teq_sb[:], in_=teq_d[:])
                tek_sb = p1w.tile([NTYPE, 1536], BF16, tag="tek")
                nc.sync.dma_start(out=tek_sb[:], in_=tek_d[:])
                bv_sb = p1w.tile([1, D], BF16, tag="bv")
                nc.sync.dma_start(out=bv_sb[:], in_=bv_d[:])
                r128_sb = p1w.tile([128, 128], F32R, tag="r128")
                nc.sync.dma_start(out=r128_sb[:], in_=r128_d[:])
                b4_sb = p1w.tile([4, 128], F32R, tag="b4")
                nc.sync.dma_start(out=b4_sb[:], in_=b4_d[:])
                invf_sb = p1w.tile([128, 1], F32, tag="invf")
                nc.sync.dma_start(out=invf_sb[:], in_=invf_d[:])
                pos4_sb = p1w.tile([4, TPC], F32R, tag="pos4")
                nc.sync.dma_start(out=pos4_sb[:], in_=pos4_d[:])
                qt_sb = p1w.tile([1, TPC], F32R, tag="qt")
                nc.sync.dma_start(out=qt_sb[:], in_=qtype_d[:])
                kt_sb = p1w.tile([1, TPC], F32R, tag="kt")
                nc.sync.dma_start(out=kt_sb[:], in_=ktype_d[:])

                # LN1
                a1, be1 = layernorm_stats(p1t, [x[:] for x in xT], "l1")
                xn = ln_apply(p1w, p1t, [x[:].bitcast(F32) for x in xT],
                              a1, be1, "l1a", out_dt=BF16)

                # one-hot type codes (16, TPC)
                p1misc = p1.enter_context(
                    tc.tile_pool(name="p1misc", bufs=1, space="PSUM"))

                def onehot(row_sb, tag):
                    bc = p1misc.tile([16, TPC], F32, tag="ohbc")
                    nc.tensor.matmul(bc[:], ones_sb[:, 0:16], row_sb[:],
                                     start=True, stop=True)
                    oh = p1w.tile([16, TPC], BF16, tag=f"{tag}oh")
                    nc.vector.tensor_scalar(out=oh[:], in0=bc[:],
                                            scalar1=iota_sb[:], scalar2=None,
                                            op0=alu.is_equal)
                    return oh
                oh_q = onehot(qt_sb, "q")
                oh_k = onehot(kt_sb, "k")

                # cos/sin tiles (128, TPC): rows 0:64 q-axes, 64:128 k-axes
                pm = p1misc.tile([128, TPC], F32, tag="pm")
                nc.tensor.matmul(pm[:], b4_sb[:], pos4_sb[:], start=True,
                                 stop=True)
                f_t = p1t.tile([128, TPC], F32, tag="f")
                nc.vector.tensor_scalar(out=f_t[:], in0=pm[:],
                                        scalar1=invf_sb[:], scalar2=None,
                                        op0=alu.mult)
                nt = p1t.tile([128, TPC], F32, tag="nt")
                nc.vector.tensor_scalar(out=nt[:], in0=f_t[:],
                                        scalar1=float(1.0 / (2 * np.pi)),
                                        scalar2=None, op0=alu.mult)
                n_i = p1t.tile([128, TPC], I32, tag="ni")
                nc.vector.tensor_copy(out=n_i[:], in_=nt[:])
                n_f = p1t.tile([128, TPC], F32, tag="nf")
                nc.vector.tensor_copy(out=n_f[:], in_=n_i[:])
                fr = p1t.tile([128, TPC], F32, tag="fr")
                nc.vector.scalar_tensor_tensor(out=fr[:], in0=n_f[:],
                                               scalar=-C1, in1=f_t[:],
                                               op0=alu.mult, op1=alu.add)
                nc.vector.scalar_tensor_tensor(out=fr[:], in0=n_f[:],
                                               scalar=-C2, in1=fr[:],
                                               op0=alu.mult, op1=alu.add)
                nc.vector.scalar_tensor_tensor(out=fr[:], in0=n_f[:],
                                               scalar=-C3, in1=fr[:],
                                               op0=alu.mult, op1=alu.add)
                s_t = p1w.tile([128, TPC], F32, tag="sin")
                nc.scalar.activation(out=s_t[:], in_=fr[:], func=act.Sin)
                af = p1t.tile([128, TPC], F32, tag="af")
                nc.scalar.activation(out=af[:], in_=fr[:], func=act.Abs)
                ca = p1t.tile([128, TPC], F32, tag="ca")
                nc.vector.tensor_scalar(out=ca[:], in0=af[:], scalar1=-1.0,
                                        scalar2=HALF_PI, op0=alu.mult,
                                        op1=alu.add)
                c_t = p1w.tile([128, TPC], F32, tag="cos")
                nc.scalar.activation(out=c_t[:], in_=ca[:], func=act.Sin)
                # fold score scale 1/sqrt(HD) into q: scale c,s rows 0:64
                nc.vector.tensor_scalar(out=c_t[0:64, :], in0=c_t[0:64, :],
                                        scalar1=float(SCALE), scalar2=None,
                                        op0=alu.mult)
                nc.vector.tensor_scalar(out=s_t[0:64, :], in0=s_t[0:64, :],
                                        scalar1=float(SCALE), scalar2=None,
                                        op0=alu.mult)

                # qk slices: matmul + type emb, then rope / tails -> slab
                for s in range(12):
                    qk_ps = p1qk.tile([128, TPC], F32, tag="qkps")
                    for k in range(KD):
                        nc.tensor.matmul(
                            qk_ps[:],
                            wqk_sb[:, 1536 * k + 128 * s:1536 * k + 128 * (s + 1)],
                            xn[k][:], start=(k == 0), stop=False)
                    nc.tensor.matmul(qk_ps[:],
                                     teq_sb[:, 128 * s:128 * (s + 1)],
                                     oh_q[:], start=False, stop=False)
                    nc.tensor.matmul(qk_ps[:],
                                     tek_sb[:, 128 * s:128 * (s + 1)],
                                     oh_k[:], start=False, stop=True)
                    if s < 8:
                        # rope: q_h 0:64 | k_h 0:64
                        rsb = p1t.tile([128, TPC], F32R, tag="rsb")
                        nc.scalar.activation(out=rsb[:], in_=qk_ps[:],
                                             func=act.Copy)
                        rot = p1qk.tile([128, TPC], F32, tag="rot")
                        nc.tensor.matmul(rot[:], r128_sb[:], rsb[:],
                                         start=True, stop=True)
                        t1 = p1t.tile([128, TPC], F32, tag="rt1")
                        nc.vector.tensor_tensor(out=t1[:],
                                                in0=rsb[:].bitcast(F32),
                                                in1=c_t[:], op=alu.mult)
                        t2 = p1t.tile([128, TPC], F32, tag="rt2")
                        nc.vector.tensor_tensor(out=t2[:], in0=rot[:],
                                                in1=s_t[:], op=alu.mult)
                        qkr = p1t.tile([128, TPC], BF16, tag="qkr")
                        nc.vector.tensor_tensor(out=qkr[:], in0=t1[:],
                                                in1=t2[:], op=alu.add)
                        h = s
                        nc.scalar.dma_start(
                            out=slab_in[288 * h + 0:288 * h + 64, :],
                            in_=qkr[0:64, :])
                        nc.scalar.dma_start(
                            out=slab_in[288 * h + 96:288 * h + 160, :],
                            in_=qkr[64:128, :])
                    else:
                        # tails: s=8,9 q tails h0..3/h4..7 (scale by 1/sqrt(HD))
                        # s=10,11 k tails
                        tl = p1t.tile([128, TPC], BF16, tag="tail")
                        sc = float(SCALE) if s < 10 else 1.0
                        nc.vector.tensor_scalar(out=tl[:], in0=qk_ps[:],
                                                scalar1=sc, scalar2=None,
                                                op0=alu.mult)
                        base = 64 if s < 10 else 160  # q tail at +64, k at +160
                        for j in range(4):
                            h = 4 * (s % 2) + j
                            nc.scalar.dma_start(
                                out=slab_in[288 * h + base:288 * h + base + 32, :],
                                in_=tl[32 * j:32 * (j + 1), :])

                # v (token-major): 4 tok-slices x 2 halves of 384 cols
                for ts_ in range(4):
                    for hf in range(2):
                        v_ps = p1qk.tile([128, 384], F32, tag="vps")
                        for k in range(KD):
                            nc.tensor.matmul(
                                v_ps[:],
                                xn[k][:, 128 * ts_:128 * (ts_ + 1)],
                                wv_sb[:, D * k + 384 * hf:D * k + 384 * (hf + 1)],
                                start=(k == 0), stop=False)
                        nc.tensor.matmul(
                            v_ps[:], onesrowb_sb[:],
                            bv_sb[:, 384 * hf:384 * (hf + 1)],
                            start=False, stop=True)
                        v_sb1 = p1t.tile([128, 384], BF16, tag="vsb1")
                        nc.vector.tensor_copy(out=v_sb1[:], in_=v_ps[:])
                        # one batched DMA: (tok, head j, dim) -> v region of
                        # blocks h=4*hf+j at flat (288h+192)*TPC + 128*ts_*96
                        dst = bass.AP(
                            tensor=slab_in[:].tensor,
                            offset=(288 * 4 * hf + 192) * TPC + 128 * ts_ * 96,
                            ap=[[96, 128], [288 * TPC, 4], [1, 96]])
                        nc.sync.dma_start(out=dst, in_=v_sb1[:])

                nc.gpsimd.collective_compute(
                    "AllToAll", mybir.AluOpType.bypass,
                    replica_groups=[list(range(N_CORES))],
                    ins=[slab_in[:].opt()],
                    outs=[slab_out[:].opt()])

            # ================= PHASE 2 =================
            # pool for FFN weights: spans phases 2+3 only
            wff_ctx = contextlib.ExitStack()
            wff = wff_ctx.enter_context(tc.tile_pool(name="wff", bufs=1))
            with contextlib.ExitStack() as p2:
                p2w = p2.enter_context(tc.tile_pool(name="p2w", bufs=1))
                p2t = p2.enter_context(tc.tile_pool(name="p2t", bufs=3))
                p2ps = p2.enter_context(
                    tc.tile_pool(name="p2ps", bufs=4, space="PSUM"))
                p2o = p2.enter_context(
                    tc.tile_pool(name="p2o", bufs=2, space="PSUM"))
                p2rb = p2.enter_context(
                    tc.tile_pool(name="p2rb", bufs=2, space="PSUM"))

                masks_sb = p2w.tile([128, 4 * 512], BF16, tag="masks")
                nc.sync.dma_start(out=masks_sb[:], in_=masks_d[:])

                # FFN weights + biases: tiles allocated now, but the loads are
                # issued inside batch 1 (scalar-queue position guarantees the
                # qkv collective has drained, keeping its DMA rings quiet)
                w1_sb = wff.tile([128, KD * 2 * DFF], BF16, tag="w1")
                w2_sb = wff.tile([128, 16 * D], BF16, tag="w2")
                b1a_sb = wff.tile([128, 16], F32, tag="b1a")
                b1g_sb = wff.tile([128, 16], F32, tag="b1g")
                bf2_sb = wff.tile([128, KD], F32, tag="bf2")

                for bb_ in range(2):
                    if bb_ == 1:
                        nc.scalar.dma_start(out=w1_sb[:], in_=w1_d[:])
                        nc.scalar.dma_start(out=w2_sb[:], in_=w2_d[:])
                        nc.scalar.dma_start(out=b1a_sb[:], in_=b1a_d[:])
                        nc.scalar.dma_start(out=b1g_sb[:], in_=b1g_d[:])
                        nc.scalar.dma_start(out=bf2_sb[:], in_=bf2_d[:])
                    qT = p2w.tile([96, 2048], BF16, tag=f"qT{bb_}")
                    kT = p2w.tile([96, 2048], BF16, tag=f"kT{bb_}")
                    v_sb = p2w.tile([128, 16, 97], BF16, tag=f"v{bb_}")
                    ones_bc = bass.AP(
                        tensor=onescolb_sb[:].tensor,
                        offset=onescolb_sb[:].offset,
                        ap=[[1, 128], [0, 16], [0, 1]])
                    nc.sync.dma_start(out=v_sb[:, :, 96:97], in_=ones_bc)
                    for u in range(4):
                        blk = 288 * (4 * bb_ + u)
                        nc.sync.dma_start(
                            out=qT[:, 512 * u:512 * (u + 1)],
                            in_=slab_out[blk + 0:blk + 96, :])
                        nc.sync.dma_start(
                            out=kT[:, 512 * u:512 * (u + 1)],
                            in_=slab_out[blk + 96:blk + 192, :])
                    # v: one gather DMA per source core (tok, tok-slice, dim)
                    vfull = v_sb[:]
                    for u in range(4):
                        nc.sync.dma_start(
                            out=bass.AP(tensor=vfull.tensor,
                                        offset=vfull.offset + 97 * 4 * u,
                                        ap=[list(vfull.ap[0]), [97, 4],
                                            [1, 96]]),
                            in_=bass.AP(
                                tensor=slab_out[:].tensor,
                                offset=(288 * (4 * bb_ + u) + 192) * TPC,
                                ap=[[96, 128], [128 * 96, 4], [1, 96]]))

                    for Q in reversed(range(NSUP)):
                        o_ps = p2o.tile([97, 512], F32, tag="ops", name="ops")
                        nkt = 4 * Q + 4
                        for kt in range(nkt):
                            s_ps = p2ps.tile([128, 512], F32, tag="sps",
                                             name="sps")
                            nc.tensor.matmul(
                                s_ps[:], kT[:, 128 * kt:128 * (kt + 1)],
                                qT[:, 512 * Q:512 * (Q + 1)],
                                start=True, stop=True)
                            e_sb = p2t.tile([128, 512], BF16, tag="esb",
                                            name="esb")
                            nc.scalar.activation(out=e_sb[:], in_=s_ps[:],
                                                 func=act.Exp)
                            dj = kt - 4 * Q
                            if dj >= 0:
                                nc.vector.tensor_tensor(
                                    out=e_sb[:], in0=e_sb[:],
                                    in1=masks_sb[:, 512 * dj:512 * (dj + 1)],
                                    op=alu.mult)
                            nc.tensor.matmul(o_ps[:], v_sb[:, kt, :], e_sb[:],
                                             start=(kt == 0),
                                             stop=(kt == nkt - 1))
                        # per-unit softmax normalization, pipelined behind
                        # the next unit's matmuls
                        j = 4 * bb_ + Q
                        rec = p2t.tile([1, 512], F32R, tag="rec")
                        with nc.allow_low_precision(reason="softmax denom"):
                            nc.vector.reciprocal(out=rec[:],
                                                 in_=o_ps[96:97, :])
                        rb_ps = p2rb.tile([96, 512], F32, tag="rb")
                        nc.tensor.matmul(rb_ps[:], ones_sb[:, 0:96], rec[:],
                                         start=True, stop=True)
                        rb_sb = p2t.tile([96, 512], F32, tag="rbsb")
                        nc.vector.tensor_copy(out=rb_sb[:], in_=rb_ps[:])
                        onrm = p2t.tile([96, 512], BF16, tag="onrm")
                        nc.vector.tensor_tensor(out=onrm[:],
                                                in0=o_ps[0:96, :],
                                                in1=rb_sb[:], op=alu.mult)
                        nc.scalar.dma_start(
                            out=slab2_in[96 * j:96 * (j + 1), :], in_=onrm[:])

            nc.gpsimd.collective_compute(
                "AllToAll", mybir.AluOpType.bypass,
                replica_groups=[list(range(N_CORES))],
                ins=[slab2_in[:].opt()], outs=[slab2_out[:].opt()])

            # ================= PHASE 3 =================
            with contextlib.ExitStack() as p3:
                p3w = p3.enter_context(tc.tile_pool(name="p3w", bufs=1))
                p3t = p3.enter_context(tc.tile_pool(name="p3t", bufs=2))

                x2 = []
                for k in range(KD):
                    o_sb = p3t.tile([128, TPC], BF16, tag="osb")
                    nc.sync.dma_start(out=o_sb[:],
                                      in_=slab2_out[128 * k:128 * (k + 1), :])
                    t = p3w.tile([128, TPC], F32R, tag=f"x2_{k}")
                    nc.vector.tensor_tensor(out=t[:], in0=o_sb[:],
                                            in1=xT[k][:].bitcast(F32),
                                            op=alu.add)
                    x2.append(t)

                a2, be2 = layernorm_stats(p3t, [t[:] for t in x2], "l2")
                x2n = ln_apply(p3w, p3t, [t[:].bitcast(F32) for t in x2],
                               a2, be2, "l2a", out_dt=BF16)

                # fc1 from prefetched packed w1 tile
                a_tiles = []
                sw = []
                with tc.tile_pool(name="p3h", bufs=2, space="PSUM") as p3h:
                    for g in range(8):           # g<4: a-half, g>=4: gate-half
                        for mi in range(4):
                            i = 4 * (g % 4) + mi
                            col = 512 * g + 128 * mi
                            h_ps = p3h.tile([128, TPC], F32, tag="hps")
                            for k in range(KD):
                                nc.tensor.matmul(
                                    h_ps[:],
                                    w1_sb[:, 4096 * k + col:4096 * k + col + 128],
                                    x2n[k][:],
                                    start=(k == 0), stop=(k == KD - 1))
                            if g < 4:
                                a_sb = p3w.tile([128, TPC], BF16, tag=f"a{i}")
                                nc.vector.tensor_scalar(
                                    out=a_sb[:], in0=h_ps[:],
                                    scalar1=b1a_sb[:, i:i + 1],
                                    scalar2=None, op0=alu.add)
                                a_tiles.append(a_sb)
                            else:
                                sil = p3t.tile([128, TPC], BF16, tag="sil")
                                nc.scalar.activation(
                                    out=sil[:], in_=h_ps[:], func=act.Silu,
                                    bias=b1g_sb[:, i:i + 1])
                                swt = p3w.tile([128, TPC], BF16, tag=f"sw{i}")
                                nc.vector.tensor_tensor(
                                    out=swt[:], in0=sil[:],
                                    in1=a_tiles[i][:], op=alu.mult)
                                sw.append(swt)

                # fc2: k2-outer, 6 persistent ff psum banks, prefetched w2
                with tc.tile_pool(name="p3f", bufs=1, space="PSUM") as p3f:
                    ff_ps = [p3f.tile([128, TPC], F32, tag=f"ff{d}",
                                      name=f"ff{d}")
                             for d in range(KD)]
                    for k2 in range(16):
                        for d in range(KD):
                            nc.tensor.matmul(
                                ff_ps[d][:],
                                w2_sb[:, D * k2 + 128 * d:D * k2 + 128 * (d + 1)],
                                sw[k2][:],
                                start=(k2 == 0), stop=(k2 == 15))
                    for d in range(KD):
                        t = p3t.tile([128, TPC], F32, tag="fft")
                        nc.vector.tensor_scalar(out=t[:], in0=ff_ps[d][:],
                                                scalar1=bf2_sb[:, d:d + 1],
                                                scalar2=None, op0=alu.add)
                        o = p3t.tile([128, TPC], F32, tag="oout")
                        nc.vector.tensor_tensor(out=o[:], in0=t[:],
                                                in1=x2[d][:].bitcast(F32),
                                                op=alu.add)
                        nc.sync.dma_start(
                            out=outT_d[128 * d:128 * (d + 1), :], in_=o[:])
            wff_ctx.close()

    nc.compile()
    _prog_cache[key] = nc
    return nc


def _host_inputs(x_type, x_value, seq_order, W_attn, type_emb, g1, b1, g2, b2,
                 W_fc1, b_fc1, W_fc2, b_fc2):
    f32 = np.float32
    x_type = np.asarray(x_type)
    seq_order = np.asarray(seq_order)
    x_value = np.asarray(x_value, dtype=f32)
    W_attn = np.asarray(W_attn, dtype=f32)
    type_emb = np.asarray(type_emb, dtype=f32)
    W_fc1 = np.asarray(W_fc1, dtype=f32)
    W_fc2 = np.asarray(W_fc2, dtype=f32)
    g1 = np.asarray(g1, f32); b1 = np.asarray(b1, f32)
    g2 = np.asarray(g2, f32); b2 = np.asarray(b2, f32)
    b_fc1 = np.asarray(b_fc1, f32); b_fc2 = np.asarray(b_fc2, f32)

    # fold LN1 gain into W_attn, LN1 bias row into type emb / v bias row
    Wqk_s = g1[:, None] * W_attn[:, :1536]
    Wv_s = g1[:, None] * W_attn[:, 1536:]
    bqk_row = b1 @ W_attn[:, :1536]          # (1536,)
    bv_row = (b1 @ W_attn[:, 1536:]).reshape(1, D)

    def pack_k(w):
        # (KD*128, C) -> (128, KD*C): chunk k at cols [C*k : C*(k+1)]
        c = w.shape[1]
        return np.ascontiguousarray(
            w.reshape(KD, 128, c).transpose(1, 0, 2).reshape(128, KD * c))

    wqk_full = pack_k(Wqk_s[:, QK_PERM]).astype(BF16NP)
    te_full = type_emb[:, QK_PERM] + bqk_row[QK_PERM][None, :]
    q_origin = QK_PERM < 768
    te_q = np.where(q_origin[None, :], te_full, 0.0).astype(BF16NP)
    te_k = np.where(~q_origin[None, :], te_full, 0.0).astype(BF16NP)

    invf16 = (1.0 / THETA ** (np.arange(0, DR, 2, dtype=f32) / DR)).astype(f32)
    invf_col = invf16[(np.arange(128) % 32) // 2].reshape(128, 1)

    # masks: block (128k x 512q), mask[kk, qq] = 1 if qq >= kk + 128*dj
    kk = np.arange(128)[:, None]
    qq = np.arange(512)[None, :]
    masks = np.concatenate(
        [(qq >= kk + 128 * dj).astype(f32) for dj in range(4)],
        axis=1).astype(BF16NP)

    # rot lhsT: lhsT[k, m] = P[m, k];  P[2i, 2i+1] = -1, P[2i+1, 2i] = +1
    R = np.zeros((128, 128), f32)
    for i in range(64):
        R[2 * i + 1, 2 * i] = -1.0
        R[2 * i, 2 * i + 1] = 1.0
    B4m = np.zeros((4, 128), f32)
    B4m[0, 0:32] = 1.0; B4m[1, 32:64] = 1.0
    B4m[2, 64:96] = 1.0; B4m[3, 96:128] = 1.0

    # fold LN2 gain/bias into W_fc1 / its bias
    W1_s = (g2[:, None] * W_fc1).astype(BF16NP)
    b_fc1_eff = b_fc1 + b2 @ W_fc1

    # W2 packed over its 16 contraction chunks: (2048, 768) -> (128, 16*768)
    W2_p = np.ascontiguousarray(
        W_fc2.reshape(16, 128, D).transpose(1, 0, 2).reshape(128, 16 * D))

    common = {
        "Wqk": wqk_full, "Wv": pack_k(Wv_s).astype(BF16NP),
        "te_q": te_q, "te_k": te_k, "bvrow": bv_row.astype(BF16NP),
        "invf": invf_col,
        "W1": pack_k(W1_s), "W2": W2_p.astype(BF16NP),
        "b1a": b_fc1_eff[:2048].reshape(16, 128).T.copy(),
        "b1g": b_fc1_eff[2048:].reshape(16, 128).T.copy(),
        "bf2": b_fc2.reshape(6, 128).T.copy(),
        "masks": masks, "R128": R, "B4": B4m,
        "ones128": np.ones((1, 128), f32),
        "onescol": np.ones((128, 1), f32),
        "onescolb": np.ones((128, 1), BF16NP),
        "onesrowb": np.ones((1, 128), BF16NP),
        "iota16": np.arange(16, dtype=f32).reshape(16, 1),
        "epsc": np.full((1, 1), EPS, f32),
    }
    in_maps = []
    for c in range(N_CORES):
        b = c // 4
        t0 = 512 * (c % 4)
        m = dict(common)
        m["xT"] = np.ascontiguousarray(x_value[b, t0:t0 + TPC, :].T)
        m["qtype"] = x_type[b, t0:t0 + TPC].astype(f32).reshape(1, TPC)
        m["ktype"] = x_type[b, t0 + 1:t0 + TPC + 1].astype(f32).reshape(1, TPC)
        pos4 = np.stack([
            seq_order[0, b, t0:t0 + TPC],
            seq_order[1, b, t0:t0 + TPC],
            seq_order[0, b, t0 + 1:t0 + TPC + 1],
            seq_order[1, b, t0 + 1:t0 + TPC + 1],
        ]).astype(f32)
        m["pos4"] = pos4
        in_maps.append(m)
    return in_maps


def kernel(**inputs):
    nc = build_program()
    in_maps = _host_inputs(**inputs)
    res = run_bass_kernel_spmd(nc, in_maps, list(range(N_CORES)), trace=False)
    out = np.empty((B, T, D), np.float32)
    for c in range(N_CORES):
        b = c // 4
        t0 = 512 * (c % 4)
        out[b, t0:t0 + TPC, :] = res.results[c]["outT"].T
    return out


# revision 45
# speedup vs baseline: 1.1450x; 1.1362x over previous
"""Trainium2 Bass kernel for nn_DecoderLayer_90967407329666.

Decoder layer: LN1 -> QKV (+type emb) -> multi-axis RoPE -> causal SDPA
-> residual -> LN2 -> SwiGLU FFN -> residual.  B=2, T=2048, D=768, H=8,
DFF=2048, NTYPE=16, NAX=2 rotary axes of 32 dims each.

Sharding (8 cores):
  Phase 1 (token-parallel): each core owns 512 tokens (core c: batch c//4,
    tokens 512*(c%4)...) and computes LN1 + q,k (feature-major) + v
    (token-major) + type-emb + RoPE for those tokens, all 8 heads.
  Single AllToAll: block h carries head h's q,k,v (288 rows bf16) ->
    core c ends up with head c for ALL 4096 tokens.
  Phase 2 (head-parallel): core c runs full causal attention for head c,
    both batches; per-unit softmax normalization pipelined behind matmuls.
  AllToAll #2 (bf16): o goes back token-parallel.
  Phase 3 (token-parallel): residual + LN2 + SwiGLU FFN + residual.

dtypes: weights + exchange slabs in bf16; residual stream, LN stats and
position/angle math in fp32/fp32r (fp32r matmuls run at full PE rate for
free-dim >= 256, same as bf16).  LN gains/biases are folded into the
weights host-side (W' = g*W, bias row b@W folded into type-emb / biases).
A tiny AllToAll issued at t=0 absorbs the cross-core rendezvous skew.
"""

import sys

sys.path.insert(0, "/opt/trn_rl_repo")

import numpy as np
import ml_dtypes

import contextlib

import concourse.bacc as bacc
import concourse.bass as bass
import concourse.tile as tile
from concourse import mybir
from concourse.bass_utils import run_bass_kernel_spmd

BF16NP = np.dtype(ml_dtypes.bfloat16)

# ---- problem constants (hardcoded per contest rules) ----
B, T = 2, 2048
D, H, DFF, NTYPE = 768, 8, 2048, 16
NAX = 2
HD = D // H            # 96
DR = HD // (NAX + 1)   # 32
EPS = 1e-5
THETA = 10000.0
N_CORES = 8
TPC = 512              # tokens per core
NSUP = 4               # supertiles per batch (2048/512)
KD = D // 128          # 6 contraction chunks over D
SCALE = 1.0 / np.sqrt(np.float32(HD))

F32 = mybir.dt.float32
F32R = mybir.dt.float32r
BF16 = mybir.dt.bfloat16
I32 = mybir.dt.int32

# Cody-Waite split of 2*pi (C1 has 12 mantissa bits -> n*C1 exact for n<2^11)
C1 = float(np.float32(np.floor(2 * np.pi * 2**9) / 2**9))
C2 = float(np.float32(2 * np.pi - C1))
C3 = float(np.float32(2 * np.pi - C1 - float(np.float32(2 * np.pi - C1))))
HALF_PI = float(np.pi / 2)

# qk output-feature permutation: 12 slices of 128 rows
#   slices 0..7  : [q_h dims 0:64 | k_h dims 0:64]   (rope rows)
#   slice  8, 9  : q tails (dims 64:96) of heads 0..3 / 4..7
#   slice 10,11  : k tails of heads 0..3 / 4..7
def _qk_colperm():
    cols = []
    for h in range(H):
        cols += list(range(96 * h, 96 * h + 64))          # q_h 0:64
        cols += list(range(768 + 96 * h, 768 + 96 * h + 64))  # k_h 0:64
    for h in range(H):
        cols += list(range(96 * h + 64, 96 * h + 96))     # q tails
    for h in range(H):
        cols += list(range(768 + 96 * h + 64, 768 + 96 * h + 96))  # k tails
    return np.array(cols)

QK_PERM = _qk_colperm()

# merged slab layout: per head h (288 rows x TPC cols, bf16):
#   rows 288h+  0.. 96 : q head h, feature-major (rope dims 0:64, tail 64:96)
#   rows 288h+ 96..192 : k head h, feature-major
#   rows 288h+192..288 : v head h, token-major packed (4 x (128tok x 96) flat)
SLABR = 288 * H  # 2304

_prog_cache = {}


def build_program():
    key = 0
    if key in _prog_cache:
        return _prog_cache[key]
    nc = bacc.Bacc("TRN2", target_bir_lowering=False, debug=False,
                   num_devices=N_CORES)
    alu = mybir.AluOpType
    act = mybir.ActivationFunctionType

    # ---------------- DRAM I/O ----------------
    xT_d = nc.dram_tensor("xT", [D, TPC], F32R, kind="ExternalInput")
    wqk_d = nc.dram_tensor("Wqk", [128, KD * 1536], BF16,
                           kind="ExternalInput")
    wv_d = nc.dram_tensor("Wv", [128, KD * D], BF16, kind="ExternalInput")
    teq_d = nc.dram_tensor("te_q", [NTYPE, 1536], BF16, kind="ExternalInput")
    tek_d = nc.dram_tensor("te_k", [NTYPE, 1536], BF16, kind="ExternalInput")
    bv_d = nc.dram_tensor("bvrow", [1, D], BF16, kind="ExternalInput")
    qtype_d = nc.dram_tensor("qtype", [1, TPC], F32R, kind="ExternalInput")
    ktype_d = nc.dram_tensor("ktype", [1, TPC], F32R, kind="ExternalInput")
    pos4_d = nc.dram_tensor("pos4", [4, TPC], F32R, kind="ExternalInput")
    invf_d = nc.dram_tensor("invf", [128, 1], F32, kind="ExternalInput")
    w1_d = nc.dram_tensor("W1", [128, KD * 2 * DFF], BF16,
                          kind="ExternalInput")
    w2_d = nc.dram_tensor("W2", [128, 16 * D], BF16, kind="ExternalInput")
    b1a_d = nc.dram_tensor("b1a", [128, 16], F32, kind="ExternalInput")
    b1g_d = nc.dram_tensor("b1g", [128, 16], F32, kind="ExternalInput")
    bf2_d = nc.dram_tensor("bf2", [128, KD], F32, kind="ExternalInput")
    masks_d = nc.dram_tensor("masks", [128, 4 * 512], BF16,
                             kind="ExternalInput")
    r128_d = nc.dram_tensor("R128", [128, 128], F32R, kind="ExternalInput")
    onesrowb_d = nc.dram_tensor("onesrowb", [1, 128], BF16,
                                kind="ExternalInput")
    b4_d = nc.dram_tensor("B4", [4, 128], F32R, kind="ExternalInput")
    ones_d = nc.dram_tensor("ones128", [1, 128], F32R, kind="ExternalInput")
    onescol_d = nc.dram_tensor("onescol", [128, 1], F32R, kind="ExternalInput")
    onescolb_d = nc.dram_tensor("onescolb", [128, 1], BF16,
                                kind="ExternalInput")
    iota_d = nc.dram_tensor("iota16", [16, 1], F32, kind="ExternalInput")
    eps_d = nc.dram_tensor("epsc", [1, 1], F32, kind="ExternalInput")
    outT_d = nc.dram_tensor("outT", [D, TPC], F32, kind="ExternalOutput")

    with tile.TileContext(nc) as tc:
        with tc.tile_pool(name="glob", bufs=1) as glob, \
             tc.tile_pool(name="dram", bufs=1, space="DRAM") as dram:
            # exchange slabs
            slab_in = dram.tile([SLABR, TPC], BF16, tag="slab_in")
            slab_out = dram.tile([SLABR, TPC], BF16, tag="slab_out")
            slab2_in = dram.tile([D, TPC], BF16, tag="slab2_in")
            slab2_out = dram.tile([D, TPC], BF16, tag="slab2_out")
            # ---- persistent constants / activations ----
            ones_sb = glob.tile([1, 128], F32R, tag="ones")
            nc.sync.dma_start(out=ones_sb[:], in_=ones_d[:])
            onescol_sb = glob.tile([128, 1], F32R, tag="onescol")
            nc.sync.dma_start(out=onescol_sb[:], in_=onescol_d[:])
            onescolb_sb = glob.tile([128, 1], BF16, tag="onescolb")
            nc.sync.dma_start(out=onescolb_sb[:], in_=onescolb_d[:])
            onesrowb_sb = glob.tile([1, 128], BF16, tag="onesrowb")
            nc.sync.dma_start(out=onesrowb_sb[:], in_=onesrowb_d[:])
            iota_sb = glob.tile([16, 1], F32, tag="iota")
            nc.sync.dma_start(out=iota_sb[:], in_=iota_d[:])
            eps_sb = glob.tile([1, 1], F32, tag="eps")
            nc.sync.dma_start(out=eps_sb[:], in_=eps_d[:])
            xT = []
            for k in range(KD):
                t = glob.tile([128, TPC], F32R, tag=f"xT{k}")
                nc.sync.dma_start(out=t[:], in_=xT_d[128 * k:128 * (k + 1), :])
                xT.append(t)

            def layernorm_stats(pool, src_tiles, tag):
                """src (fp32r aps, 6 x (128,TPC)) -> (alpha_row, beta_row)."""
                ctx = contextlib.ExitStack()
                ps_pool = ctx.enter_context(
                    tc.tile_pool(name=f"{tag}ps", bufs=1, space="PSUM"))
                sums = ps_pool.tile([1, TPC], F32, tag=f"{tag}sums")
                sumsq = ps_pool.tile([1, TPC], F32, tag=f"{tag}sumsq")
                for k in range(KD):
                    sq = pool.tile([128, TPC], F32R, tag=f"{tag}sq")
                    nc.scalar.activation(out=sq[:],
                                         in_=src_tiles[k].bitcast(F32),
                                         func=act.Square)
                    nc.tensor.matmul(sums[:], onescol_sb[:], src_tiles[k],
                                     start=(k == 0), stop=(k == KD - 1))
                    nc.tensor.matmul(sumsq[:], onescol_sb[:], sq[:],
                                     start=(k == 0), stop=(k == KD - 1))
                mean = pool.tile([1, TPC], F32R, tag=f"{tag}mean")
                nc.vector.tensor_scalar(out=mean[:], in0=sums[:],
                                        scalar1=1.0 / D, scalar2=None,
                                        op0=alu.mult)
                m2 = pool.tile([1, TPC], F32, tag=f"{tag}m2")
                nc.vector.tensor_tensor(out=m2[:],
                                        in0=mean[:].bitcast(F32),
                                        in1=mean[:].bitcast(F32), op=alu.mult)
                var = pool.tile([1, TPC], F32, tag=f"{tag}var")
                nc.vector.scalar_tensor_tensor(
                    out=var[:], in0=sumsq[:], scalar=1.0 / D, in1=m2[:],
                    op0=alu.mult, op1=alu.subtract)
                std = pool.tile([1, TPC], F32, tag=f"{tag}std")
                nc.scalar.activation(out=std[:], in_=var[:], func=act.Sqrt,
                                     bias=eps_sb[:])
                al_f = pool.tile([1, TPC], F32, tag=f"{tag}alf")
                nc.vector.reciprocal_approx_fast(out=al_f[:], in_=std[:])
                alpha = pool.tile([1, TPC], F32R, tag=f"{tag}alpha")
                nc.vector.tensor_copy(out=alpha[:], in_=al_f[:])
                beta = pool.tile([1, TPC], F32R, tag=f"{tag}beta")
                nc.vector.scalar_tensor_tensor(
                    out=beta[:], in0=mean[:].bitcast(F32), scalar=-1.0,
                    in1=al_f[:], op0=alu.mult, op1=alu.mult)
                ctx.close()
                return alpha, beta

            def ln_apply(pool, scratch, src_f32_aps, alpha, beta, tag,
                         out_dt=F32R):
                """out[k] = src*alpha_bcast + beta_bcast (g/b folded into W)."""
                ctx = contextlib.ExitStack()
                ps_pool = ctx.enter_context(
                    tc.tile_pool(name=f"{tag}ps", bufs=1, space="PSUM"))
                ab = ps_pool.tile([128, TPC], F32, tag=f"{tag}ab")
                nc.tensor.matmul(ab[:], ones_sb[:], alpha[:], start=True,
                                 stop=True)
                bb = ps_pool.tile([128, TPC], F32, tag=f"{tag}bb")
                nc.tensor.matmul(bb[:], ones_sb[:], beta[:], start=True,
                                 stop=True)
                ab_sb = pool.tile([128, TPC], F32, tag=f"{tag}absb")
                nc.vector.tensor_copy(out=ab_sb[:], in_=ab[:])
                bb_sb = pool.tile([128, TPC], F32, tag=f"{tag}bbsb")
                nc.vector.tensor_copy(out=bb_sb[:], in_=bb[:])
                ctx.close()
                outs = []
                for k in range(KD):
                    t1 = scratch.tile([128, TPC], F32, tag=f"{tag}t1")
                    nc.vector.tensor_tensor(out=t1[:], in0=src_f32_aps[k],
                                            in1=ab_sb[:], op=alu.mult)
                    o = pool.tile([128, TPC], out_dt, tag=f"{tag}o{k}")
                    nc.vector.tensor_tensor(out=o[:], in0=t1[:], in1=bb_sb[:],
                                            op=alu.add)
                    outs.append(o)
                return outs

            # ================= PHASE 1 =================
            with contextlib.ExitStack() as p1:
                p1w = p1.enter_context(tc.tile_pool(name="p1w", bufs=1))
                p1t = p1.enter_context(tc.tile_pool(name="p1t", bufs=2))
                p1qk = p1.enter_context(
                    tc.tile_pool(name="p1qk", bufs=2, space="PSUM"))

                wqk_sb = p1w.tile([128, KD * 1536], BF16, tag="wqk")
                nc.sync.dma_start(out=wqk_sb[:], in_=wqk_d[:])
                wv_sb = p1w.tile([128, KD * D], BF16, tag="wv")
                nc.sync.dma_start(out=wv_sb[:], in_=wv_d[:])
                teq_sb = p1w.tile([NTYPE, 1536], BF16, tag="teq")
                nc.sync.dma_start(out=teq_sb[:], in_=teq_d[:])
                tek_sb = p1w.tile([NTYPE, 1536], BF16, tag="tek")
                nc.sync.dma_start(out=tek_sb[:], in_=tek_d[:])
                bv_sb = p1w.tile([1, D], BF16, tag="bv")
                nc.sync.dma_start(out=bv_sb[:], in_=bv_d[:])
                r128_sb = p1w.tile([128, 128], F32R, tag="r128")
                nc.sync.dma_start(out=r128_sb[:], in_=r128_d[:])
                b4_sb = p1w.tile([4, 128], F32R, tag="b4")
                nc.sync.dma_start(out=b4_sb[:], in_=b4_d[:])
                invf_sb = p1w.tile([128, 1], F32, tag="invf")
                nc.sync.dma_start(out=invf_sb[:], in_=invf_d[:])
                pos4_sb = p1w.tile([4, TPC], F32R, tag="pos4")
                nc.sync.dma_start(out=pos4_sb[:], in_=pos4_d[:])
                qt_sb = p1w.tile([1, TPC], F32R, tag="qt")
                nc.sync.dma_start(out=qt_sb[:], in_=qtype_d[:])
                kt_sb = p1w.tile([1, TPC], F32R, tag="kt")
                nc.sync.dma_start(out=kt_sb[:], in_=ktype_d[:])

                # LN1
                a1, be1 = layernorm_stats(p1t, [x[:] for x in xT], "l1")
                xn = ln_apply(p1w, p1t, [x[:].bitcast(F32) for x in xT],
                              a1, be1, "l1a", out_dt=BF16)

                # one-hot type codes (16, TPC)
                p1misc = p1.enter_context(
                    tc.tile_pool(name="p1misc", bufs=1, space="PSUM"))

                def onehot(row_sb, tag):
                    bc = p1misc.tile([16, TPC], F32, tag="ohbc")
                    nc.tensor.matmul(bc[:], ones_sb[:, 0:16], row_sb[:],
                                     start=True, stop=True)
                    oh = p1w.tile([16, TPC], BF16, tag=f"{tag}oh")
                    nc.vector.tensor_scalar(out=oh[:], in0=bc[:],
                                            scalar1=iota_sb[:], scalar2=None,
                                            op0=alu.is_equal)
                    return oh
                oh_q = onehot(qt_sb, "q")
                oh_k = onehot(kt_sb, "k")

                # cos/sin tiles (128, TPC): rows 0:64 q-axes, 64:128 k-axes
                pm = p1misc.tile([128, TPC], F32, tag="pm")
                nc.tensor.matmul(pm[:], b4_sb[:], pos4_sb[:], start=True,
                                 stop=True)
                f_t = p1t.tile([128, TPC], F32, tag="f")
                nc.vector.tensor_scalar(out=f_t[:], in0=pm[:],
                                        scalar1=invf_sb[:], scalar2=None,
                                        op0=alu.mult)
                nt = p1t.tile([128, TPC], F32, tag="nt")
                nc.vector.tensor_scalar(out=nt[:], in0=f_t[:],
                                        scalar1=float(1.0 / (2 * np.pi)),
                                        scalar2=None, op0=alu.mult)
                n_i = p1t.tile([128, TPC], I32, tag="ni")
                nc.vector.tensor_copy(out=n_i[:], in_=nt[:])
                n_f = p1t.tile([128, TPC], F32, tag="nf")
                nc.vector.tensor_copy(out=n_f[:], in_=n_i[:])
                fr = p1t.tile([128, TPC], F32, tag="fr")
                nc.vector.cody_waite_cascade(out=fr[:], x=f_t[:], k=n_f[:],
                                             c1=C1, c2=C2, c3=C3)
                s_t = p1w.tile([128, TPC], F32, tag="sin")
                nc.scalar.activation(out=s_t[:], in_=fr[:], func=act.Sin)
                af = p1t.tile([128, TPC], F32, tag="af")
                nc.scalar.activation(out=af[:], in_=fr[:], func=act.Abs)
                ca = p1t.tile([128, TPC], F32, tag="ca")
                nc.vector.tensor_scalar(out=ca[:], in0=af[:], scalar1=-1.0,
                                        scalar2=HALF_PI, op0=alu.mult,
                                        op1=alu.add)
                c_t = p1w.tile([128, TPC], F32, tag="cos")
                nc.scalar.activation(out=c_t[:], in_=ca[:], func=act.Sin)
                # fold score scale 1/sqrt(HD) into q: scale c,s rows 0:64
                nc.vector.tensor_scalar(out=c_t[0:64, :], in0=c_t[0:64, :],
                                        scalar1=float(SCALE), scalar2=None,
                                        op0=alu.mult)
                nc.vector.tensor_scalar(out=s_t[0:64, :], in0=s_t[0:64, :],
                                        scalar1=float(SCALE), scalar2=None,
                                        op0=alu.mult)

                # qk slices: matmul + type emb, then rope / tails -> slab
                for s in range(12):
                    qk_ps = p1qk.tile([128, TPC], F32, tag="qkps")
                    for k in range(KD):
                        nc.tensor.matmul(
                            qk_ps[:],
                            wqk_sb[:, 1536 * k + 128 * s:1536 * k + 128 * (s + 1)],
                            xn[k][:], start=(k == 0), stop=False)
                    nc.tensor.matmul(qk_ps[:],
                                     teq_sb[:, 128 * s:128 * (s + 1)],
                                     oh_q[:], start=False, stop=False)
                    nc.tensor.matmul(qk_ps[:],
                                     tek_sb[:, 128 * s:128 * (s + 1)],
                                     oh_k[:], start=False, stop=True)
                    if s < 8:
                        # rope: q_h 0:64 | k_h 0:64
                        rsb = p1t.tile([128, TPC], F32R, tag="rsb")
                        nc.scalar.activation(out=rsb[:], in_=qk_ps[:],
                                             func=act.Copy)
                        rot = p1qk.tile([128, TPC], F32, tag="rot")
                        nc.tensor.matmul(rot[:], r128_sb[:], rsb[:],
                                         start=True, stop=True)
                        t1 = p1t.tile([128, TPC], F32, tag="rt1")
                        nc.vector.tensor_tensor(out=t1[:],
                                                in0=rsb[:].bitcast(F32),
                                                in1=c_t[:], op=alu.mult)
                        t2 = p1t.tile([128, TPC], F32, tag="rt2")
                        nc.vector.tensor_tensor(out=t2[:], in0=rot[:],
                                                in1=s_t[:], op=alu.mult)
                        qkr = p1t.tile([128, TPC], BF16, tag="qkr")
                        nc.vector.tensor_tensor(out=qkr[:], in0=t1[:],
                                                in1=t2[:], op=alu.add)
                        h = s
                        nc.scalar.dma_start(
                            out=slab_in[288 * h + 0:288 * h + 64, :],
                            in_=qkr[0:64, :])
                        nc.scalar.dma_start(
                            out=slab_in[288 * h + 96:288 * h + 160, :],
                            in_=qkr[64:128, :])
                    else:
                        # tails: s=8,9 q tails h0..3/h4..7 (scale by 1/sqrt(HD))
                        # s=10,11 k tails
                        tl = p1t.tile([128, TPC], BF16, tag="tail")
                        sc = float(SCALE) if s < 10 else 1.0
                        nc.vector.tensor_scalar(out=tl[:], in0=qk_ps[:],
                                                scalar1=sc, scalar2=None,
                                                op0=alu.mult)
                        base = 64 if s < 10 else 160  # q tail at +64, k at +160
                        for j in range(4):
                            h = 4 * (s % 2) + j
                            nc.scalar.dma_start(
                                out=slab_in[288 * h + base:288 * h + base + 32, :],
                                in_=tl[32 * j:32 * (j + 1), :])

                # v (token-major): 4 tok-slices x 2 halves of 384 cols
                for ts_ in range(4):
                    for hf in range(2):
                        v_ps = p1qk.tile([128, 384], F32, tag="vps")
                        for k in range(KD):
                            nc.tensor.matmul(
                                v_ps[:],
                                xn[k][:, 128 * ts_:128 * (ts_ + 1)],
                                wv_sb[:, D * k + 384 * hf:D * k + 384 * (hf + 1)],
                                start=(k == 0), stop=False)
                        nc.tensor.matmul(
                            v_ps[:], onesrowb_sb[:],
                            bv_sb[:, 384 * hf:384 * (hf + 1)],
                            start=False, stop=True)
                        v_sb1 = p1t.tile([128, 384], BF16, tag="vsb1")
                        nc.vector.tensor_copy(out=v_sb1[:], in_=v_ps[:])
                        # one batched DMA: (tok, head j, dim) -> v region of
                        # blocks h=4*hf+j at flat (288h+192)*TPC + 128*ts_*96
                        dst = bass.AP(
                            tensor=slab_in[:].tensor,
                            offset=(288 * 4 * hf + 192) * TPC + 128 * ts_ * 96,
                            ap=[[96, 128], [288 * TPC, 4], [1, 96]])
                        nc.sync.dma_start(out=dst, in_=v_sb1[:])

                nc.gpsimd.collective_compute(
                    "AllToAll", mybir.AluOpType.bypass,
                    replica_groups=[list(range(N_CORES))],
                    ins=[slab_in[:].opt()],
                    outs=[slab_out[:].opt()])

            # ================= PHASE 2 =================
            # pool for FFN weights: spans phases 2+3 only
            wff_ctx = contextlib.ExitStack()
            wff = wff_ctx.enter_context(tc.tile_pool(name="wff", bufs=1))
            with contextlib.ExitStack() as p2:
                p2w = p2.enter_context(tc.tile_pool(name="p2w", bufs=1))
                p2t = p2.enter_context(tc.tile_pool(name="p2t", bufs=3))
                p2ps = p2.enter_context(
                    tc.tile_pool(name="p2ps", bufs=4, space="PSUM"))
                p2o = p2.enter_context(
                    tc.tile_pool(name="p2o", bufs=2, space="PSUM"))
                p2rb = p2.enter_context(
                    tc.tile_pool(name="p2rb", bufs=2, space="PSUM"))

                masks_sb = p2w.tile([128, 4 * 512], BF16, tag="masks")
                nc.sync.dma_start(out=masks_sb[:], in_=masks_d[:])

                # FFN weights + biases: tiles allocated now, but the loads are
                # issued inside batch 1 (scalar-queue position guarantees the
                # qkv collective has drained, keeping its DMA rings quiet)
                w1_sb = wff.tile([128, KD * 2 * DFF], BF16, tag="w1")
                w2_sb = wff.tile([128, 16 * D], BF16, tag="w2")
                b1a_sb = wff.tile([128, 16], F32, tag="b1a")
                b1g_sb = wff.tile([128, 16], F32, tag="b1g")
                bf2_sb = wff.tile([128, KD], F32, tag="bf2")

                for bb_ in range(2):
                    if bb_ == 1:
                        nc.scalar.dma_start(out=w1_sb[:], in_=w1_d[:])
                        nc.scalar.dma_start(out=w2_sb[:], in_=w2_d[:])
                        nc.scalar.dma_start(out=b1a_sb[:], in_=b1a_d[:])
                        nc.scalar.dma_start(out=b1g_sb[:], in_=b1g_d[:])
                        nc.scalar.dma_start(out=bf2_sb[:], in_=bf2_d[:])
                    qT = p2w.tile([96, 2048], BF16, tag=f"qT{bb_}")
                    kT = p2w.tile([96, 2048], BF16, tag=f"kT{bb_}")
                    v_sb = p2w.tile([128, 16, 97], BF16, tag=f"v{bb_}")
                    ones_bc = bass.AP(
                        tensor=onescolb_sb[:].tensor,
                        offset=onescolb_sb[:].offset,
                        ap=[[1, 128], [0, 16], [0, 1]])
                    nc.sync.dma_start(out=v_sb[:, :, 96:97], in_=ones_bc)
                    for u in range(4):
                        blk = 288 * (4 * bb_ + u)
                        nc.sync.dma_start(
                            out=qT[:, 512 * u:512 * (u + 1)],
                            in_=slab_out[blk + 0:blk + 96, :])
                        nc.sync.dma_start(
                            out=kT[:, 512 * u:512 * (u + 1)],
                            in_=slab_out[blk + 96:blk + 192, :])
                    # v: one gather DMA per source core (tok, tok-slice, dim)
                    vfull = v_sb[:]
                    for u in range(4):
                        nc.sync.dma_start(
                            out=bass.AP(tensor=vfull.tensor,
                                        offset=vfull.offset + 97 * 4 * u,
                                        ap=[list(vfull.ap[0]), [97, 4],
                                            [1, 96]]),
                            in_=bass.AP(
                                tensor=slab_out[:].tensor,
                                offset=(288 * (4 * bb_ + u) + 192) * TPC,
                                ap=[[96, 128], [128 * 96, 4], [1, 96]]))

                    for Q in reversed(range(NSUP)):
                        o_ps = p2o.tile([97, 512], F32, tag="ops", name="ops")
                        nkt = 4 * Q + 4
                        for kt in range(nkt):
                            s_ps = p2ps.tile([128, 512], F32, tag="sps",
                                             name="sps")
                            nc.tensor.matmul(
                                s_ps[:], kT[:, 128 * kt:128 * (kt + 1)],
                                qT[:, 512 * Q:512 * (Q + 1)],
                                start=True, stop=True)
                            e_sb = p2t.tile([128, 512], BF16, tag="esb",
                                            name="esb")
                            nc.scalar.activation(out=e_sb[:], in_=s_ps[:],
                                                 func=act.Exp)
                            dj = kt - 4 * Q
                            if dj >= 0:
                                nc.vector.tensor_tensor(
                                    out=e_sb[:], in0=e_sb[:],
                                    in1=masks_sb[:, 512 * dj:512 * (dj + 1)],
                                    op=alu.mult)
                            nc.tensor.matmul(o_ps[:], v_sb[:, kt, :], e_sb[:],
                                             start=(kt == 0),
                                             stop=(kt == nkt - 1))
                        # per-unit softmax normalization, pipelined behind
                        # the next unit's matmuls
                        j = 4 * bb_ + Q
                        den_sb = p2t.tile([1, 512], F32, tag="densb")
                        nc.vector.tensor_copy(out=den_sb[:],
                                              in_=o_ps[96:97, :])
                        rec_f = p2t.tile([1, 512], F32, tag="recf")
                        nc.vector.reciprocal_approx_fast(out=rec_f[:],
                                                         in_=den_sb[:])
                        rec = p2t.tile([1, 512], F32R, tag="rec")
                        nc.vector.tensor_copy(out=rec[:], in_=rec_f[:])
                        rb_ps = p2rb.tile([96, 512], F32, tag="rb")
                        nc.tensor.matmul(rb_ps[:], ones_sb[:, 0:96], rec[:],
                                         start=True, stop=True)
                        rb_sb = p2t.tile([96, 512], F32, tag="rbsb")
                        nc.vector.tensor_copy(out=rb_sb[:], in_=rb_ps[:])
                        onrm = p2t.tile([96, 512], BF16, tag="onrm")
                        nc.vector.tensor_tensor(out=onrm[:],
                                                in0=o_ps[0:96, :],
                                                in1=rb_sb[:], op=alu.mult)
                        nc.scalar.dma_start(
                            out=slab2_in[96 * j:96 * (j + 1), :], in_=onrm[:])

            nc.gpsimd.collective_compute(
                "AllToAll", mybir.AluOpType.bypass,
                replica_groups=[list(range(N_CORES))],
                ins=[slab2_in[:].opt()], outs=[slab2_out[:].opt()])

            # ================= PHASE 3 =================
            with contextlib.ExitStack() as p3:
                p3w = p3.enter_context(tc.tile_pool(name="p3w", bufs=1))
                p3t = p3.enter_context(tc.tile_pool(name="p3t", bufs=2))

                x2 = []
                for k in range(KD):
                    o_sb = p3t.tile([128, TPC], BF16, tag="osb")
                    nc.sync.dma_start(out=o_sb[:],
                                      in_=slab2_out[128 * k:128 * (k + 1), :])
                    t = p3w.tile([128, TPC], F32R, tag=f"x2_{k}")
                    nc.vector.tensor_tensor(out=t[:], in0=o_sb[:],
                                            in1=xT[k][:].bitcast(F32),
                                            op=alu.add)
                    x2.append(t)

                a2, be2 = layernorm_stats(p3t, [t[:] for t in x2], "l2")
                x2n = ln_apply(p3w, p3t, [t[:].bitcast(F32) for t in x2],
                               a2, be2, "l2a", out_dt=BF16)

                # fc1 from prefetched packed w1 tile
                a_tiles = []
                sw = []
                with tc.tile_pool(name="p3h", bufs=4, space="PSUM") as p3h:
                    for g in range(8):           # g<4: a-half, g>=4: gate-half
                        for mi in range(4):
                            i = 4 * (g % 4) + mi
                            col = 512 * g + 128 * mi
                            h_ps = p3h.tile([128, TPC], F32, tag="hps")
                            for k in range(KD):
                                nc.tensor.matmul(
                                    h_ps[:],
                                    w1_sb[:, 4096 * k + col:4096 * k + col + 128],
                                    x2n[k][:],
                                    start=(k == 0), stop=(k == KD - 1))
                            if g < 4:
                                a_sb = p3w.tile([128, TPC], BF16, tag=f"a{i}")
                                nc.scalar.activation(
                                    out=a_sb[:], in_=h_ps[:],
                                    func=act.Identity,
                                    bias=b1a_sb[:, i:i + 1])
                                a_tiles.append(a_sb)
                            else:
                                sil = p3t.tile([128, TPC], BF16, tag="sil")
                                nc.scalar.activation(
                                    out=sil[:], in_=h_ps[:], func=act.Silu,
                                    bias=b1g_sb[:, i:i + 1])
                                swt = p3w.tile([128, TPC], BF16, tag=f"sw{i}")
                                nc.vector.tensor_tensor(
                                    out=swt[:], in0=sil[:],
                                    in1=a_tiles[i][:], op=alu.mult)
                                sw.append(swt)

                # fc2: k2-outer, 6 persistent ff psum banks, prefetched w2
                with tc.tile_pool(name="p3f", bufs=1, space="PSUM") as p3f:
                    ff_ps = [p3f.tile([128, TPC], F32, tag=f"ff{d}",
                                      name=f"ff{d}")
                             for d in range(KD)]
                    for k2 in range(16):
                        for d in range(KD):
                            nc.tensor.matmul(
                                ff_ps[d][:],
                                w2_sb[:, D * k2 + 128 * d:D * k2 + 128 * (d + 1)],
                                sw[k2][:],
                                start=(k2 == 0), stop=(k2 == 15))
                    for d in range(KD):
                        t = p3t.tile([128, TPC], F32, tag="fft")
                        nc.vector.tensor_scalar(out=t[:], in0=ff_ps[d][:],
                                                scalar1=bf2_sb[:, d:d + 1],
                                                scalar2=None, op0=alu.add)
                        o = p3t.tile([128, TPC], F32, tag="oout")
                        nc.vector.tensor_tensor(out=o[:], in0=t[:],
                                                in1=x2[d][:].bitcast(F32),
                                                op=alu.add)
                        nc.sync.dma_start(
                            out=outT_d[128 * d:128 * (d + 1), :], in_=o[:])
            wff_ctx.close()

    nc.compile()
    _prog_cache[key] = nc
    return nc


def _host_inputs(x_type, x_value, seq_order, W_attn, type_emb, g1, b1, g2, b2,
                 W_fc1, b_fc1, W_fc2, b_fc2):
    f32 = np.float32
    x_type = np.asarray(x_type)
    seq_order = np.asarray(seq_order)
    x_value = np.asarray(x_value, dtype=f32)
    W_attn = np.asarray(W_attn, dtype=f32)
    type_emb = np.asarray(type_emb, dtype=f32)
    W_fc1 = np.asarray(W_fc1, dtype=f32)
    W_fc2 = np.asarray(W_fc2, dtype=f32)
    g1 = np.asarray(g1, f32); b1 = np.asarray(b1, f32)
    g2 = np.asarray(g2, f32); b2 = np.asarray(b2, f32)
    b_fc1 = np.asarray(b_fc1, f32); b_fc2 = np.asarray(b_fc2, f32)

    # fold LN1 gain into W_attn, LN1 bias row into type emb / v bias row
    Wqk_s = g1[:, None] * W_attn[:, :1536]
    Wv_s = g1[:, None] * W_attn[:, 1536:]
    bqk_row = b1 @ W_attn[:, :1536]          # (1536,)
    bv_row = (b1 @ W_attn[:, 1536:]).reshape(1, D)

    def pack_k(w):
        # (KD*128, C) -> (128, KD*C): chunk k at cols [C*k : C*(k+1)]
        c = w.shape[1]
        return np.ascontiguousarray(
            w.reshape(KD, 128, c).transpose(1, 0, 2).reshape(128, KD * c))

    wqk_full = pack_k(Wqk_s[:, QK_PERM]).astype(BF16NP)
    te_full = type_emb[:, QK_PERM] + bqk_row[QK_PERM][None, :]
    q_origin = QK_PERM < 768
    te_q = np.where(q_origin[None, :], te_full, 0.0).astype(BF16NP)
    te_k = np.where(~q_origin[None, :], te_full, 0.0).astype(BF16NP)

    invf16 = (1.0 / THETA ** (np.arange(0, DR, 2, dtype=f32) / DR)).astype(f32)
    invf_col = invf16[(np.arange(128) % 32) // 2].reshape(128, 1)

    # masks: block (128k x 512q), mask[kk, qq] = 1 if qq >= kk + 128*dj
    kk = np.arange(128)[:, None]
    qq = np.arange(512)[None, :]
    masks = np.concatenate(
        [(qq >= kk + 128 * dj).astype(f32) for dj in range(4)],
        axis=1).astype(BF16NP)

    # rot lhsT: lhsT[k, m] = P[m, k];  P[2i, 2i+1] = -1, P[2i+1, 2i] = +1
    R = np.zeros((128, 128), f32)
    for i in range(64):
        R[2 * i + 1, 2 * i] = -1.0
        R[2 * i, 2 * i + 1] = 1.0
    B4m = np.zeros((4, 128), f32)
    B4m[0, 0:32] = 1.0; B4m[1, 32:64] = 1.0
    B4m[2, 64:96] = 1.0; B4m[3, 96:128] = 1.0

    # fold LN2 gain/bias into W_fc1 / its bias
    W1_s = (g2[:, None] * W_fc1).astype(BF16NP)
    b_fc1_eff = b_fc1 + b2 @ W_fc1

    # W2 packed over its 16 contraction chunks: (2048, 768) -> (128, 16*768)
    W2_p = np.ascontiguousarray(
        W_fc2.reshape(16, 128, D).transpose(1, 0, 2).reshape(128, 16 * D))

    common = {
        "Wqk": wqk_full, "Wv": pack_k(Wv_s).astype(BF16NP),
        "te_q": te_q, "te_k": te_k, "bvrow": bv_row.astype(BF16NP),
        "invf": invf_col,
        "W1": pack_k(W1_s), "W2": W2_p.astype(BF16NP),
        "b1a": b_fc1_eff[:2048].reshape(16, 128).T.copy(),
        "b1g": b_fc1_eff[2048:].reshape(16, 128).T.copy(),
        "bf2": b_fc2.reshape(6, 128).T.copy(),
        "masks": masks, "R128": R, "B4": B4m,
        "ones128": np.ones((1, 128), f32),
        "onescol": np.ones((128, 1), f32),
        "onescolb": np.ones((128, 1), BF16NP),
        "onesrowb": np.ones((1, 128), BF16NP),
        "iota16": np.arange(16, dtype=f32).reshape(16, 1),
        "epsc": np.full((1, 1), EPS, f32),
    }
    in_maps = []
    for c in range(N_CORES):
        b = c // 4
        t0 = 512 * (c % 4)
        m = dict(common)
        m["xT"] = np.ascontiguousarray(x_value[b, t0:t0 + TPC, :].T)
        m["qtype"] = x_type[b, t0:t0 + TPC].astype(f32).reshape(1, TPC)
        m["ktype"] = x_type[b, t0 + 1:t0 + TPC + 1].astype(f32).reshape(1, TPC)
        pos4 = np.stack([
            seq_order[0, b, t0:t0 + TPC],
            seq_order[1, b, t0:t0 + TPC],
            seq_order[0, b, t0 + 1:t0 + TPC + 1],
            seq_order[1, b, t0 + 1:t0 + TPC + 1],
        ]).astype(f32)
        m["pos4"] = pos4
        in_maps.append(m)
    return in_maps


def kernel(**inputs):
    nc = build_program()
    in_maps = _host_inputs(**inputs)
    res = run_bass_kernel_spmd(nc, in_maps, list(range(N_CORES)), trace=False)
    out = np.empty((B, T, D), np.float32)
    for c in range(N_CORES):
        b = c // 4
        t0 = 512 * (c % 4)
        out[b, t0:t0 + TPC, :] = res.results[c]["outT"].T
    return out
